# revision 27
# baseline (speedup 1.0000x reference)
"""FF-sharded MoE FFN kernel for Trainium2 (8 NeuronCores), v2 "W8".

Strategy (pure FF-tensor-parallel, single group):
  - Host computes the gate in fp32 (softmax -> top-2 -> renormalize).
  - Every core processes ALL routed (expert, token) visits; the FFN
    hidden dim (FF=4096) is sharded 8 ways: core c holds columns
    [c*512, (c+1)*512) of every expert's W1 and the matching rows of
    W2, and computes
        Ypart = gelu(X @ W1[:, shard] + b1[shard]) @ W2[shard, :]
    for each expert segment. The host sums the 8 partials, applies the
    top-2 combine weights, and adds the b2 term.
  - Why: per-core work is exactly sum(counts)/8 * H * FFS MAC columns
    for ANY routing - zero load imbalance and zero slot padding (the
    previous expert-pairing scheme padded ~1%). HBM traffic is
    ~50MB/core (16 W + 17 x + 17 y), hidden under ~265us of matmul.

Per-core schedule (8 segments = experts, descending token count):
  Inputs are packed PARTITION-MAJOR: per SBUF partition, each DMA'd
  piece is one contiguous [k][col] run, so every transfer is 128 large
  descriptors (small strided descriptors measured as low as 37GB/s;
  large ones ~245GB/s). Queue assignment is driven by measured queue
  rates: the sync queue is the fast one, so ALL latency-critical
  input (w1|x) and output (y) traffic goes to sync in exact
  consumption order; w2 and b1 (needed one GEMM-phase later) ride the
  parallel gpsimd SW-DGE queue. Every dma_start costs ~2-6us of
  queue-side latency before data flows, so segment 0 uses ONE merged
  [w1 | x-block0] head DMA (~1.5MB, first data ~13us) plus one DMA per
  later x block; GEMM2 blocks interleave one-behind GEMM1 (lag-1) so
  the PE has ~2x compute per input byte while transfers land.
  Zero-matmul warm-up (NBMAX-wide, accumulated ahead of the first real
  PSUM group) covers the initial DMA wait; it must keep the PE busy
  CONTIGUOUSLY ~2 aligned 3.4us HAM windows or the PE stays at half
  clock (any idle gap resets the window).
  All GEMMs bf16 on the PE with fp32 PSUM accumulation; exact gelu is
  fused into the GEMM1 PSUM eviction (ScalarE) with the b1 bias; GEMM2
  evictions (VectorE) write bf16 into a per-block PACKED staging tile
  so the y output DMA is one contiguous run per partition. The last
  two blocks drain in staggered 2/4-row-chunk DMAs so only ~0.25MB of
  transfer trails the final matmul.
"""

import sys

if "/opt/trn_rl_repo" not in sys.path:
    sys.path.insert(0, "/opt/trn_rl_repo")

import numpy as np
import ml_dtypes

H = 1024          # hidden size
E = 8             # experts
TOPK = 2
FF = 4 * H        # expert hidden dim
P = 128           # SBUF partitions
NC = 8            # cores == FF shards
FFS = FF // NC    # per-core FF shard (512)
KH = H // P       # 8  contraction chunks for GEMM1
KFS = FFS // P    # 4  contraction chunks for GEMM2 (shard)
NB0 = 256         # segment-0 head block width (two of them)

_prog_cache: dict[tuple, object] = {}
LAST_RESULTS = None  # BassKernelResults of the most recent run (for test harness)
TRACE = False        # test harness can set kernel.TRACE = True for profiling
ACT_OVERRIDE = None  # sim-only: CoreSim lacks Gelu; tests may set e.g. "Relu"
LAST_CALL = None     # (nc, in_maps) of the most recent run, for re-runs
WARM_N = 26          # HAM/pstate pre-warm zero-matmuls at kernel start.
# Zero matmuls are NBMAX wide; they accumulate into the first PSUM group
# ahead of the real contraction. They must keep the PE busy CONTIGUOUSLY
# until the merged head DMA lands (~16us): HAM only flips to full clock
# after ~2 aligned 3.4us windows of uninterrupted busy.
# (phase, bi, group) -> zero MMs prepended to that group (stall bridging).
BRIDGE = {}


def _seg_blocks(A: int, first: int | None = None):
    """Split A token columns into near-equal blocks <= 512.

    first: width of the first TWO blocks (segment 0 only): small head
    blocks let the first GEMMs start as soon as ~0.8MB of input has
    landed. Avoid blocks < ~230: below that LDWEIGHTS (~114ns) stops
    hiding behind the matmul stream.
    """
    blocks = []
    t = 0
    if first is not None:
        nb = min(first, A)
        blocks.append((t, nb))
        t = nb
        A -= nb
    if A > 0:
        nblk = -(-A // 512)
        base = A // nblk
        rem = A % nblk
        for i in range(nblk):
            nb = base + (1 if i < rem else 0)
            blocks.append((t, nb))
            t += nb
    return blocks


def _plan(segs: tuple[int, ...]):
    """Shared host/kernel plan: per-segment blocks + packed-y offsets.

    Returns (blocks_by_seg, boffs) where boffs[(si, bi)] is the element
    offset (per partition) of that block's [ht][t]-contiguous span in
    the packed y output.
    """
    nb0 = min(NB0, segs[0])
    blocks_by_seg = [
        _seg_blocks(A, first=nb0 if si == 0 else None)
        for si, A in enumerate(segs)
    ]
    boffs = {}
    off = 0
    for si, blocks in enumerate(blocks_by_seg):
        for bi, (t0, nb) in enumerate(blocks):
            boffs[(si, bi)] = off
            off += KH * nb
    return blocks_by_seg, boffs


def _build_program(segs: tuple[int, ...], use_bias: bool = True):
    """Build + compile the per-core SPMD Bass program.

    segs: token count per segment, descending (exact per-expert counts;
    identical on all cores).

    DRAM I/O (S = len(segs), Ctot = sum(segs)):
      xw  [P, 8*(S*FFS + Ctot)] bf16  partition-major packed inputs:
          per partition, per piece: [k][cols] contiguous (segment 0 is
          stored as separate pieces: w1c0 | xblk0 | w1c1 | w1c2.. |
          xblk1 | xblk2 ..)
      w2  [P, S*KFS*H] bf16  partition-major W2 shards
      b1p [P, S*KFS]  f32   b1 shard, col si*KFS+f = b1[f*128:(f+1)*128]
      y   [P, KH*Ctot] bf16 partial YT, packed per block: each block's
          span is [ht][t] contiguous per partition (host sums cores,
          then unpacks)
    """
    from contextlib import ExitStack

    from concourse import bacc
    import concourse.mybir as mybir
    import concourse.tile as tile

    dt = mybir.dt
    S = len(segs)
    Ctot = sum(segs)
    A0 = segs[0]
    nb0 = min(NB0, A0)
    blocks_by_seg, boffs = _plan(segs)
    NBMAX = max(nb for blocks in blocks_by_seg for _, nb in blocks)

    nc = bacc.Bacc(None, target_bir_lowering=False, debug=False)

    xw = nc.dram_tensor("xw", [P, KH * (S * FFS + Ctot)], dt.bfloat16,
                        kind="ExternalInput")
    w2 = nc.dram_tensor("w2", [P, S * KFS * H], dt.bfloat16,
                        kind="ExternalInput")
    b1p = nc.dram_tensor("b1p", [P, S * KFS], dt.float32, kind="ExternalInput")
    y = nc.dram_tensor("y", [P, KH * Ctot], dt.bfloat16, kind="ExternalOutput")

    # xw element offset (per partition) of each segment's packed block;
    # segment 0 occupies [0, KH*(FFS+A0)) split into its pieces.
    seg_off = [0]
    for A in segs:
        seg_off.append(seg_off[-1] + KH * (FFS + A))

    def xw_src(elem_off: int, ncols: int):
        """2D contiguous per-partition run of xw (128 big descriptors --
        3D APs here cost ~2.6us of DGE descriptor-generation per DMA)."""
        a = elem_off
        return xw[:, a:a + KH * ncols]

    with ExitStack() as ctx:
        tc = ctx.enter_context(tile.TileContext(nc))
        xwpool = ctx.enter_context(tc.tile_pool(name="xwpool", bufs=2))
        w2pool = ctx.enter_context(tc.tile_pool(name="w2pool", bufs=2))
        bpool = ctx.enter_context(tc.tile_pool(name="bpool", bufs=1))
        hpool = ctx.enter_context(tc.tile_pool(name="hpool", bufs=2))
        psA = ctx.enter_context(tc.tile_pool(name="psA", bufs=4, space="PSUM"))
        psB = ctx.enter_context(tc.tile_pool(name="psB", bufs=4, space="PSUM"))
        opool = ctx.enter_context(tc.tile_pool(name="opool", bufs=3))

        act = getattr(mybir.ActivationFunctionType, ACT_OVERRIDE or "Gelu")
        tiles = {}

        b1t = bpool.tile([P, S * KFS], dt.float32, tag="b1t", name="b1t")

        # --- segment 0: a merged [w1 | x-block0] head tile (ONE dma --
        # each dma instruction costs ~2us of queue latency) + one tile
        # per later x block, all on sync in consumption order; w2/b1 on
        # the parallel gpsimd queue ---
        blocks0 = blocks_by_seg[0]
        head0 = bpool.tile([P, KH * (FFS + nb0)], dt.bfloat16, tag="head0",
                           name="head0")
        xts = [None] + [bpool.tile([P, KH * nb], dt.bfloat16, tag=f"xt{bi}",
                                   name=f"xt{bi}")
                        for bi, (t0, nb) in enumerate(blocks0[1:], 1)]

        def emit_seg0():
            # tiny probe DMA first: absorbs the sync queue's first-use
            # setup cost (~6.5us measured) so head0's data flows sooner
            probe = bpool.tile([P, 1], dt.float32, tag="probe", name="probe")
            nc.sync.dma_start(out=probe[:, :], in_=b1p[:, 0:1])
            nc.sync.dma_start(out=head0[:, :], in_=xw_src(0, FFS + nb0))
            o = KH * (FFS + nb0)
            emit_w2(0)
            if use_bias:
                nc.gpsimd.dma_start(out=b1t[:], in_=b1p[:, :])
            for bi in range(1, len(blocks0)):
                nb = blocks0[bi][1]
                nc.sync.dma_start(out=xts[bi][:, :], in_=xw_src(o, nb))
                o += KH * nb

        def emit_w2(si):
            w2t = w2pool.tile([P, KFS * H], dt.bfloat16, tag="w2t",
                              name=f"w2t{si}")
            tiles[("w2", si)] = w2t
            nc.gpsimd.dma_start(
                out=w2t[:, :], in_=w2[:, si * KFS * H:(si + 1) * KFS * H])

        def lhsT2(si, w2t, k, ht):
            """GEMM2 stationary operand: w2 ht-chunk (128 cols)."""
            a = k * H + ht * P
            return w2t[:, a:a + P]

        def emit_seg(si):
            A = segs[si]
            ct = xwpool.tile([P, KH * (FFS + A)], dt.bfloat16, tag="ct",
                             name=f"ct{si}")
            tiles[("ct", si)] = ct
            nc.sync.dma_start(out=ct[:, :], in_=xw_src(seg_off[si], FFS + A))
            emit_w2(si)

        def lhsT1(si, k, ff):
            """GEMM1 stationary operand: w1 ff-chunk (128 cols)."""
            if si == 0:
                a = k * (FFS + nb0) + ff * P
                return head0[:, a:a + P]
            ct = tiles[("ct", si)]
            A = segs[si]
            a = k * (FFS + A) + ff * P
            return ct[:, a:a + P]

        def rhs1(si, k, bi, t0, nb):
            """GEMM1 moving operand: x token block."""
            if si == 0:
                if bi == 0:
                    a = k * (FFS + nb0) + FFS
                    return head0[:, a:a + nb]
                nb_bi = blocks_by_seg[0][bi][1]
                a = k * nb_bi
                return xts[bi][:, a:a + nb]
            ct = tiles[("ct", si)]
            A = segs[si]
            a = k * (FFS + A) + FFS + t0
            return ct[:, a:a + nb]

        # warm-up zero tile first in the vector queue (no input deps) so
        # the PE can start ramping before any DMA lands
        warm = bpool.tile([P, NBMAX], dt.bfloat16, tag="warm", name="warm")
        nc.vector.memset(warm[:, :], 0.0)

        emit_seg0()
        if S > 1:
            emit_seg(1)

        def zero_pad(ps, nb, n):
            """n zero matmuls accumulated into a PSUM group (busy filler).

            Full NBMAX width regardless of nb: wider zeros cover more
            wall-clock per instruction; the real accumulation only reads
            ps[:, :nb].
            """
            for i in range(n):
                nc.tensor.matmul(
                    ps[:, :NBMAX],
                    lhsT=warm[:, :P],
                    rhs=warm[:, :NBMAX],
                    start=(i == 0),
                    stop=False,
                )

        def g1_block(si, hblk, bi, t0, nb):
            """GEMM1 for one token block -> hblk[:, :, t0:t0+nb]."""
            for ff in range(KFS):
                pa = psA.tile([P, NBMAX], dt.float32, tag="pa",
                              name=f"pa{si}_{bi}_{ff}")
                warm_n = 0
                if si == 0 and bi == 0 and ff == 0:
                    # Pre-warm: accumulate zero-matmuls into the first
                    # PSUM group while the first input DMAs land; also
                    # ramps the PE clock out of its cold p-state.
                    warm_n = WARM_N
                elif si == 0:
                    warm_n = BRIDGE.get(("g1", bi, ff), 0)
                zero_pad(pa, nb, warm_n)
                for k in range(KH):
                    nc.tensor.matmul(
                        pa[:, :nb],
                        lhsT=lhsT1(si, k, ff),
                        rhs=rhs1(si, k, bi, t0, nb),
                        start=(k == 0 and warm_n == 0),
                        stop=(k == KH - 1),
                    )
                if use_bias:
                    nc.scalar.activation(
                        hblk[:, ff, t0:t0 + nb],
                        pa[:, :nb],
                        act,
                        bias=b1t[:, si * KFS + ff:si * KFS + ff + 1],
                    )
                else:
                    nc.scalar.activation(
                        hblk[:, ff, t0:t0 + nb],
                        pa[:, :nb],
                        act,
                    )

        def g2_block(si, w2t, hblk, bi, t0, nb, last_seg_blocks):
            """GEMM2 for one token block -> packed y DMA."""
            boff = boffs[(si, bi)]
            # ot is PACKED at stride nb so the output DMA is one
            # contiguous [ht][t] run per partition (large descriptors)
            ot = opool.tile([P, KH * NBMAX], dt.bfloat16, tag="ot",
                            name=f"ot{si}_{bi}")
            # tail: the last two blocks drain in staggered row-chunks
            # so only a small transfer trails the final matmul
            nblk_left = last_seg_blocks - bi if si == S - 1 else 99
            if nblk_left == 1:       # final block: 2-ht then 1-ht chunks
                stagger = {1: 0, 3: 2, 5: 4, 6: 6, 7: 7}
            elif nblk_left == 2:     # second-to-last: 4-ht chunks
                stagger = {3: 0, 7: 4}
            else:
                stagger = None
            for ht in range(KH):
                pb = psB.tile([P, NBMAX], dt.float32, tag="pb",
                              name=f"pb{si}_{bi}_{ht}")
                warm_n = BRIDGE.get(("g2", bi, ht), 0) if si == 0 else 0
                zero_pad(pb, nb, warm_n)
                for k in range(KFS):
                    nc.tensor.matmul(
                        pb[:, :nb],
                        lhsT=lhsT2(si, w2t, k, ht),
                        rhs=hblk[:, k, t0:t0 + nb],
                        start=(k == 0 and warm_n == 0),
                        stop=(k == KFS - 1),
                    )
                nc.vector.tensor_copy(ot[:, ht * nb:(ht + 1) * nb],
                                      pb[:, :nb])
                if stagger is not None and ht in stagger:
                    lo = stagger[ht]
                    nc.sync.dma_start(
                        out=y[:, boff + lo * nb:boff + (ht + 1) * nb],
                        in_=ot[:, lo * nb:(ht + 1) * nb],
                    )
            if stagger is None:
                nc.sync.dma_start(
                    out=y[:, boff:boff + KH * nb],
                    in_=ot[:, :KH * nb],
                )

        for si, A in enumerate(segs):
            blocks = blocks_by_seg[si]
            nblk = len(blocks)
            hblk = hpool.tile([P, KFS, A], dt.bfloat16, tag="hblk",
                              name=f"hblk{si}")
            if si == 0:
                # Segment 0 is DMA-arrival-paced: interleave GEMM2 blocks
                # one behind GEMM1 (lag-1) so the PE has ~2x compute per
                # input byte while the head transfers land.
                w2t = tiles.pop(("w2", 0))
                for i in range(nblk + 1):
                    if i >= 1:
                        t0, nb = blocks[i - 1]
                        g2_block(0, w2t, hblk, i - 1, t0, nb, nblk)
                    if i < nblk:
                        t0, nb = blocks[i]
                        g1_block(0, hblk, i, t0, nb)
            else:
                for bi, (t0, nb) in enumerate(blocks):
                    g1_block(si, hblk, bi, t0, nb)
                    if bi == 0 and si + 1 < S:
                        # Prefetch segment si+1 while the rest of this
                        # segment computes (~28us of cover for ~4MB).
                        emit_seg(si + 1)
                w2t = tiles.pop(("w2", si))
                for bi, (t0, nb) in enumerate(blocks):
                    g2_block(si, w2t, hblk, bi, t0, nb, nblk)

    nc.compile()
    return nc


def _get_program(segs: tuple[int, ...], use_bias: bool = True):
    key = (segs, use_bias)
    if key not in _prog_cache:
        _prog_cache[key] = _build_program(segs, use_bias)
    return _prog_cache[key]


def _route(xf: np.ndarray, Wg: np.ndarray, bg: np.ndarray):
    """fp32 gate: softmax -> top-2 (stable order, matches jax top_k) -> renorm."""
    logits = xf @ np.asarray(Wg, np.float32) + np.asarray(bg, np.float32)
    m = logits.max(axis=1, keepdims=True)
    p = np.exp(logits - m, dtype=np.float32)
    p /= p.sum(axis=1, keepdims=True)
    order = np.argsort(-p, axis=1, kind="stable")
    idx = order[:, :TOPK]
    pv = np.take_along_axis(p, idx, axis=1)
    vals = (pv / pv.sum(axis=1, keepdims=True)).astype(np.float32)
    return idx, vals


def _pack_pm(arr_hc: np.ndarray) -> np.ndarray:
    """[H, C] -> partition-major [P, KH*C] (per partition: [k][c])."""
    h, c = arr_hc.shape
    return np.ascontiguousarray(
        arr_hc.reshape(h // P, P, c).transpose(1, 0, 2).reshape(P, -1)
    )


def kernel(x, Wg, bg, W1, b1, W2, b2):
    global LAST_RESULTS, LAST_CALL
    from concourse.bass_utils import run_bass_kernel_spmd

    bf16 = ml_dtypes.bfloat16
    x = np.asarray(x, np.float32)
    xf = x.reshape(-1, H)
    T = xf.shape[0]

    idx, vals = _route(xf, Wg, bg)
    counts = np.bincount(idx.ravel(), minlength=E)

    # Segments: experts by token count (desc), zero-count experts skipped.
    order = [int(e) for e in np.argsort(-counts, kind="stable") if counts[e] > 0]
    segs = tuple(int(counts[e]) for e in order)
    S = len(segs)
    Ctot = sum(segs)
    A0 = segs[0]
    nb0 = min(NB0, A0)
    blocks_by_seg, boffs = _plan(segs)

    use_bias = bool(np.any(np.asarray(b1, np.float32)))
    nc = _get_program(segs, use_bias)

    W1 = np.asarray(W1, np.float32)
    W2 = np.asarray(W2, np.float32)
    b1 = np.asarray(b1, np.float32)

    # Token ids / combine scales / packed x^T per segment (shared by cores).
    shards = []
    xparts = []   # per segment: [P, KH*A] partition-major bf16
    for si in range(S):
        e = order[si]
        sel = idx == e                  # [T, 2]; at most one True per row
        ids = np.nonzero(sel.any(axis=1))[0]
        sc = vals[sel]                  # row-major => aligned with ids
        shards.append((ids, sc))
        xparts.append(_pack_pm(xf[ids].T.astype(bf16)))

    in_maps = []
    for c in range(NC):
        pieces = []
        for si in range(S):
            e = order[si]
            w1s = W1[e][:, c * FFS:(c + 1) * FFS].astype(bf16)
            if si == 0:
                # merged head piece: per partition [k][(w1 512 | xb0)],
                # then one k-major piece per later x block
                x3 = xparts[0].reshape(P, KH, A0)
                w13 = _pack_pm(w1s).reshape(P, KH, FFS)
                pieces.append(np.concatenate([w13, x3[:, :, :nb0]], axis=2)
                              .reshape(P, -1))
                for t0, nb in blocks_by_seg[0][1:]:
                    pieces.append(np.ascontiguousarray(x3[:, :, t0:t0 + nb])
                                  .reshape(P, -1))
            else:
                # per partition: [k][w1 cols | x cols] contiguous
                w13 = _pack_pm(w1s).reshape(P, KH, FFS)
                x3 = xparts[si].reshape(P, KH, segs[si])
                pieces.append(np.concatenate([w13, x3], axis=2)
                              .reshape(P, -1))
        xwc = np.ascontiguousarray(np.concatenate(pieces, axis=1))
        w2c = np.concatenate(
            [_pack_pm(W2[order[si]][c * FFS:(c + 1) * FFS, :].astype(bf16))
             for si in range(S)],
            axis=1,
        )
        b1c = np.ascontiguousarray(np.stack(
            [b1[order[si]][c * FFS + f * P:c * FFS + (f + 1) * P]
             for si in range(S) for f in range(KFS)],
            axis=1,
        ))
        in_maps.append({"xw": xwc, "w2": np.ascontiguousarray(w2c), "b1p": b1c})

    LAST_CALL = (nc, in_maps)
    LAST_RESULTS = run_bass_kernel_spmd(nc, in_maps, list(range(NC)),
                                        trace=TRACE)

    # Sum partials across cores in the packed layout, then unpack.
    ysum2 = np.zeros((P, KH * Ctot), np.float32)
    for c in range(NC):
        ysum2 += LAST_RESULTS.results[c]["y"].astype(np.float32)
    ysum = np.empty((H, Ctot), np.float32)
    soff = 0
    for si in range(S):
        for bi, (t0, nb) in enumerate(blocks_by_seg[si]):
            boff = boffs[(si, bi)]
            blk = ysum2[:, boff:boff + KH * nb].reshape(P, KH, nb)
            ysum[:, soff + t0:soff + t0 + nb] = (
                blk.transpose(1, 0, 2).reshape(H, nb))
        soff += segs[si]

    out = np.zeros((T, H), np.float32)
    c0 = 0
    for si in range(S):
        ids, sc = shards[si]
        out[ids] += ysum[:, c0:c0 + ids.size].T * sc[:, None]
        c0 += segs[si]

    b2 = np.asarray(b2, np.float32)
    out += vals[:, 0:1] * b2[idx[:, 0]] + vals[:, 1:2] * b2[idx[:, 1]]
    return out.reshape(x.shape)


# revision 28
# speedup vs baseline: 1.0268x; 1.0268x over previous
"""FF-sharded MoE FFN kernel for Trainium2 (8 NeuronCores), v2 "W8".

Strategy (pure FF-tensor-parallel, single group):
  - Host computes the gate in fp32 (softmax -> top-2 -> renormalize).
  - Every core processes ALL routed (expert, token) visits; the FFN
    hidden dim (FF=4096) is sharded 8 ways: core c holds columns
    [c*512, (c+1)*512) of every expert's W1 and the matching rows of
    W2, and computes
        Ypart = gelu(X @ W1[:, shard] + b1[shard]) @ W2[shard, :]
    for each expert segment. The host sums the 8 partials, applies the
    top-2 combine weights, and adds the b2 term.
  - Why: per-core work is exactly sum(counts)/8 * H * FFS MAC columns
    for ANY routing - zero load imbalance and zero slot padding (the
    previous expert-pairing scheme padded ~1%). HBM traffic is
    ~50MB/core (16 W + 17 x + 17 y), hidden under ~265us of matmul.

Per-core schedule (8 segments = experts, descending token count):
  Inputs are packed PARTITION-MAJOR: per SBUF partition, each DMA'd
  piece is one contiguous [k][col] run, so every transfer is 128 large
  descriptors (small strided descriptors measured as low as 37GB/s;
  large ones ~245GB/s). Queue assignment is driven by measured queue
  rates: the sync queue is the fast one, so ALL latency-critical
  input (w1|x) and output (y) traffic goes to sync in exact
  consumption order; w2 and b1 (needed one GEMM-phase later) ride the
  parallel gpsimd SW-DGE queue. Every dma_start costs ~2-6us of
  queue-side latency before data flows, so segment 0 uses ONE merged
  [w1 | x-block0] head DMA (~1.5MB, first data ~13us) plus one DMA per
  later x block; GEMM2 blocks interleave one-behind GEMM1 (lag-1) so
  the PE has ~2x compute per input byte while transfers land.
  Zero-matmul warm-up (NBMAX-wide, accumulated ahead of the first real
  PSUM group) covers the initial DMA wait; it must keep the PE busy
  CONTIGUOUSLY ~2 aligned 3.4us HAM windows or the PE stays at half
  clock (any idle gap resets the window).
  All GEMMs bf16 on the PE with fp32 PSUM accumulation; exact gelu is
  fused into the GEMM1 PSUM eviction (ScalarE) with the b1 bias; GEMM2
  evictions (VectorE) write bf16 into a per-block PACKED staging tile
  so the y output DMA is one contiguous run per partition. The last
  two blocks drain in staggered 2/4-row-chunk DMAs so only ~0.25MB of
  transfer trails the final matmul.
"""

import sys

if "/opt/trn_rl_repo" not in sys.path:
    sys.path.insert(0, "/opt/trn_rl_repo")

import numpy as np
import ml_dtypes

H = 1024          # hidden size
E = 8             # experts
TOPK = 2
FF = 4 * H        # expert hidden dim
P = 128           # SBUF partitions
NC = 8            # cores == FF shards
FFS = FF // NC    # per-core FF shard (512)
KH = H // P       # 8  contraction chunks for GEMM1
KFS = FFS // P    # 4  contraction chunks for GEMM2 (shard)
NB0 = 256         # segment-0 head block width (two of them)

_prog_cache: dict[tuple, object] = {}
LAST_RESULTS = None  # BassKernelResults of the most recent run (for test harness)
TRACE = False        # test harness can set kernel.TRACE = True for profiling
ACT_OVERRIDE = None  # sim-only: CoreSim lacks Gelu; tests may set e.g. "Relu"
LAST_CALL = None     # (nc, in_maps) of the most recent run, for re-runs
WARM_N = 26          # HAM/pstate pre-warm zero-matmuls at kernel start.
# Zero matmuls are NBMAX wide; they accumulate into the first PSUM group
# ahead of the real contraction. They must keep the PE busy CONTIGUOUSLY
# until the merged head DMA lands (~16us): HAM only flips to full clock
# after ~2 aligned 3.4us windows of uninterrupted busy.
# (phase, bi, group) -> zero MMs prepended to that group (stall bridging).
BRIDGE = {}


def _seg_blocks(A: int, first: int | None = None):
    """Split A token columns into near-equal blocks <= 512.

    first: width of the first TWO blocks (segment 0 only): small head
    blocks let the first GEMMs start as soon as ~0.8MB of input has
    landed. Avoid blocks < ~230: below that LDWEIGHTS (~114ns) stops
    hiding behind the matmul stream.
    """
    blocks = []
    t = 0
    if first is not None:
        nb = min(first, A)
        blocks.append((t, nb))
        t = nb
        A -= nb
    if A > 0:
        nblk = -(-A // 512)
        base = A // nblk
        rem = A % nblk
        for i in range(nblk):
            nb = base + (1 if i < rem else 0)
            blocks.append((t, nb))
            t += nb
    return blocks


def _plan(segs: tuple[int, ...]):
    """Shared host/kernel plan: per-segment blocks + packed-y offsets.

    Returns (blocks_by_seg, boffs) where boffs[(si, bi)] is the element
    offset (per partition) of that block's [ht][t]-contiguous span in
    the packed y output.
    """
    nb0 = min(NB0, segs[0])
    blocks_by_seg = [
        _seg_blocks(A, first=nb0 if si == 0 else None)
        for si, A in enumerate(segs)
    ]
    boffs = {}
    off = 0
    for si, blocks in enumerate(blocks_by_seg):
        for bi, (t0, nb) in enumerate(blocks):
            boffs[(si, bi)] = off
            off += KH * nb
    return blocks_by_seg, boffs


def _build_program(segs: tuple[int, ...], use_bias: bool = True):
    """Build + compile the per-core SPMD Bass program.

    segs: token count per segment, descending (exact per-expert counts;
    identical on all cores).

    DRAM I/O (S = len(segs), Ctot = sum(segs)):
      xw  [P, 8*(S*FFS + Ctot)] bf16  partition-major packed inputs:
          per partition, per piece: [k][cols] contiguous (segment 0 is
          stored as separate pieces: w1c0 | xblk0 | w1c1 | w1c2.. |
          xblk1 | xblk2 ..)
      w2  [P, S*KFS*H] bf16  partition-major W2 shards
      b1p [P, S*KFS]  f32   b1 shard, col si*KFS+f = b1[f*128:(f+1)*128]
      y   [P, KH*Ctot] bf16 partial YT, packed per block: each block's
          span is [ht][t] contiguous per partition (host sums cores,
          then unpacks)
    """
    from contextlib import ExitStack

    from concourse import bacc
    import concourse.mybir as mybir
    import concourse.tile as tile

    dt = mybir.dt
    S = len(segs)
    Ctot = sum(segs)
    A0 = segs[0]
    nb0 = min(NB0, A0)
    blocks_by_seg, boffs = _plan(segs)
    NBMAX = max(nb for blocks in blocks_by_seg for _, nb in blocks)

    nc = bacc.Bacc(None, target_bir_lowering=False, debug=False)

    xw = nc.dram_tensor("xw", [P, KH * (S * FFS + Ctot)], dt.bfloat16,
                        kind="ExternalInput")
    w2 = nc.dram_tensor("w2", [P, S * KFS * H], dt.bfloat16,
                        kind="ExternalInput")
    b1p = nc.dram_tensor("b1p", [P, S * KFS], dt.float32, kind="ExternalInput")
    y = nc.dram_tensor("y", [P, KH * Ctot], dt.bfloat16, kind="ExternalOutput")

    # xw element offset (per partition) of each segment's packed block;
    # segment 0 occupies [0, KH*(FFS+A0)) split into its pieces.
    seg_off = [0]
    for A in segs:
        seg_off.append(seg_off[-1] + KH * (FFS + A))

    def xw_src(elem_off: int, ncols: int):
        """2D contiguous per-partition run of xw (128 big descriptors --
        3D APs here cost ~2.6us of DGE descriptor-generation per DMA)."""
        a = elem_off
        return xw[:, a:a + KH * ncols]

    with ExitStack() as ctx:
        tc = ctx.enter_context(tile.TileContext(nc))
        xwpool = ctx.enter_context(tc.tile_pool(name="xwpool", bufs=2))
        w2pool = ctx.enter_context(tc.tile_pool(name="w2pool", bufs=2))
        bpool = ctx.enter_context(tc.tile_pool(name="bpool", bufs=1))
        hpool = ctx.enter_context(tc.tile_pool(name="hpool", bufs=2))
        psA = ctx.enter_context(tc.tile_pool(name="psA", bufs=4, space="PSUM"))
        psB = ctx.enter_context(tc.tile_pool(name="psB", bufs=4, space="PSUM"))
        opool = ctx.enter_context(tc.tile_pool(name="opool", bufs=3))

        act = getattr(mybir.ActivationFunctionType, ACT_OVERRIDE or "Gelu")
        tiles = {}

        b1t = bpool.tile([P, S * KFS], dt.float32, tag="b1t", name="b1t")

        # --- segment 0: a merged [w1 | x-block0] head tile (ONE dma --
        # each dma instruction costs ~2us of queue latency) + one tile
        # per later x block, all on sync in consumption order; w2/b1 on
        # the parallel gpsimd queue ---
        blocks0 = blocks_by_seg[0]
        head0 = bpool.tile([P, KH * (FFS + nb0)], dt.bfloat16, tag="head0",
                           name="head0")
        xts = [None] + [bpool.tile([P, KH * nb], dt.bfloat16, tag=f"xt{bi}",
                                   name=f"xt{bi}")
                        for bi, (t0, nb) in enumerate(blocks0[1:], 1)]

        def emit_seg0():
            nc.sync.dma_start(out=head0[:, :], in_=xw_src(0, FFS + nb0))
            o = KH * (FFS + nb0)
            emit_w2(0)
            if use_bias:
                nc.gpsimd.dma_start(out=b1t[:], in_=b1p[:, :])
            for bi in range(1, len(blocks0)):
                nb = blocks0[bi][1]
                nc.sync.dma_start(out=xts[bi][:, :], in_=xw_src(o, nb))
                o += KH * nb

        def emit_w2(si):
            w2t = w2pool.tile([P, KFS * H], dt.bfloat16, tag="w2t",
                              name=f"w2t{si}")
            tiles[("w2", si)] = w2t
            nc.gpsimd.dma_start(
                out=w2t[:, :], in_=w2[:, si * KFS * H:(si + 1) * KFS * H])

        def lhsT2(si, w2t, k, ht):
            """GEMM2 stationary operand: w2 ht-chunk (128 cols)."""
            a = k * H + ht * P
            return w2t[:, a:a + P]

        def emit_seg(si):
            A = segs[si]
            ct = xwpool.tile([P, KH * (FFS + A)], dt.bfloat16, tag="ct",
                             name=f"ct{si}")
            tiles[("ct", si)] = ct
            nc.sync.dma_start(out=ct[:, :], in_=xw_src(seg_off[si], FFS + A))
            emit_w2(si)

        def lhsT1(si, k, ff):
            """GEMM1 stationary operand: w1 ff-chunk (128 cols)."""
            if si == 0:
                a = k * (FFS + nb0) + ff * P
                return head0[:, a:a + P]
            ct = tiles[("ct", si)]
            A = segs[si]
            a = k * (FFS + A) + ff * P
            return ct[:, a:a + P]

        def rhs1(si, k, bi, t0, nb):
            """GEMM1 moving operand: x token block."""
            if si == 0:
                if bi == 0:
                    a = k * (FFS + nb0) + FFS
                    return head0[:, a:a + nb]
                nb_bi = blocks_by_seg[0][bi][1]
                a = k * nb_bi
                return xts[bi][:, a:a + nb]
            ct = tiles[("ct", si)]
            A = segs[si]
            a = k * (FFS + A) + FFS + t0
            return ct[:, a:a + nb]

        # warm-up zero tile first in the vector queue (no input deps) so
        # the PE can start ramping before any DMA lands
        warm = bpool.tile([P, NBMAX], dt.bfloat16, tag="warm", name="warm")
        nc.vector.memset(warm[:, :], 0.0)

        emit_seg0()
        if S > 1:
            emit_seg(1)

        def zero_pad(ps, nb, n):
            """n zero matmuls accumulated into a PSUM group (busy filler).

            Full NBMAX width regardless of nb: wider zeros cover more
            wall-clock per instruction; the real accumulation only reads
            ps[:, :nb].
            """
            for i in range(n):
                nc.tensor.matmul(
                    ps[:, :NBMAX],
                    lhsT=warm[:, :P],
                    rhs=warm[:, :NBMAX],
                    start=(i == 0),
                    stop=False,
                )

        def g1_block(si, hblk, bi, t0, nb):
            """GEMM1 for one token block -> hblk[:, :, t0:t0+nb]."""
            for ff in range(KFS):
                pa = psA.tile([P, NBMAX], dt.float32, tag="pa",
                              name=f"pa{si}_{bi}_{ff}")
                warm_n = 0
                if si == 0 and bi == 0 and ff == 0:
                    # Pre-warm: accumulate zero-matmuls into the first
                    # PSUM group while the first input DMAs land; also
                    # ramps the PE clock out of its cold p-state.
                    warm_n = WARM_N
                elif si == 0:
                    warm_n = BRIDGE.get(("g1", bi, ff), 0)
                zero_pad(pa, nb, warm_n)
                for k in range(KH):
                    nc.tensor.matmul(
                        pa[:, :nb],
                        lhsT=lhsT1(si, k, ff),
                        rhs=rhs1(si, k, bi, t0, nb),
                        start=(k == 0 and warm_n == 0),
                        stop=(k == KH - 1),
                    )
                if use_bias:
                    nc.scalar.activation(
                        hblk[:, ff, t0:t0 + nb],
                        pa[:, :nb],
                        act,
                        bias=b1t[:, si * KFS + ff:si * KFS + ff + 1],
                    )
                else:
                    nc.scalar.activation(
                        hblk[:, ff, t0:t0 + nb],
                        pa[:, :nb],
                        act,
                    )

        def g2_block(si, w2t, hblk, bi, t0, nb, last_seg_blocks):
            """GEMM2 for one token block -> packed y DMA."""
            boff = boffs[(si, bi)]
            # ot is PACKED at stride nb so the output DMA is one
            # contiguous [ht][t] run per partition (large descriptors)
            ot = opool.tile([P, KH * NBMAX], dt.bfloat16, tag="ot",
                            name=f"ot{si}_{bi}")
            # tail: the last two blocks drain in staggered row-chunks
            # so only a small transfer trails the final matmul
            nblk_left = last_seg_blocks - bi if si == S - 1 else 99
            if nblk_left == 1:       # final block: 2-ht then 1-ht chunks
                stagger = {1: 0, 3: 2, 5: 4, 6: 6, 7: 7}
            elif nblk_left == 2:     # second-to-last: 4-ht chunks
                stagger = {3: 0, 7: 4}
            else:
                stagger = None
            for ht in range(KH):
                pb = psB.tile([P, NBMAX], dt.float32, tag="pb",
                              name=f"pb{si}_{bi}_{ht}")
                warm_n = BRIDGE.get(("g2", bi, ht), 0) if si == 0 else 0
                zero_pad(pb, nb, warm_n)
                for k in range(KFS):
                    nc.tensor.matmul(
                        pb[:, :nb],
                        lhsT=lhsT2(si, w2t, k, ht),
                        rhs=hblk[:, k, t0:t0 + nb],
                        start=(k == 0 and warm_n == 0),
                        stop=(k == KFS - 1),
                    )
                nc.vector.tensor_copy(ot[:, ht * nb:(ht + 1) * nb],
                                      pb[:, :nb])
                if stagger is not None and ht in stagger:
                    lo = stagger[ht]
                    nc.sync.dma_start(
                        out=y[:, boff + lo * nb:boff + (ht + 1) * nb],
                        in_=ot[:, lo * nb:(ht + 1) * nb],
                    )
            if stagger is None:
                nc.sync.dma_start(
                    out=y[:, boff:boff + KH * nb],
                    in_=ot[:, :KH * nb],
                )

        for si, A in enumerate(segs):
            blocks = blocks_by_seg[si]
            nblk = len(blocks)
            hblk = hpool.tile([P, KFS, A], dt.bfloat16, tag="hblk",
                              name=f"hblk{si}")
            if si == 0:
                # Segment 0 is DMA-arrival-paced: interleave GEMM2 blocks
                # one behind GEMM1 (lag-1) so the PE has ~2x compute per
                # input byte while the head transfers land.
                w2t = tiles.pop(("w2", 0))
                for i in range(nblk + 1):
                    if i >= 1:
                        t0, nb = blocks[i - 1]
                        g2_block(0, w2t, hblk, i - 1, t0, nb, nblk)
                    if i < nblk:
                        t0, nb = blocks[i]
                        g1_block(0, hblk, i, t0, nb)
            else:
                for bi, (t0, nb) in enumerate(blocks):
                    g1_block(si, hblk, bi, t0, nb)
                    if bi == 0 and si + 1 < S:
                        # Prefetch segment si+1 while the rest of this
                        # segment computes (~28us of cover for ~4MB).
                        emit_seg(si + 1)
                w2t = tiles.pop(("w2", si))
                for bi, (t0, nb) in enumerate(blocks):
                    g2_block(si, w2t, hblk, bi, t0, nb, nblk)

    nc.compile()
    return nc


def _get_program(segs: tuple[int, ...], use_bias: bool = True):
    key = (segs, use_bias)
    if key not in _prog_cache:
        _prog_cache[key] = _build_program(segs, use_bias)
    return _prog_cache[key]


def _route(xf: np.ndarray, Wg: np.ndarray, bg: np.ndarray):
    """fp32 gate: softmax -> top-2 (stable order, matches jax top_k) -> renorm."""
    logits = xf @ np.asarray(Wg, np.float32) + np.asarray(bg, np.float32)
    m = logits.max(axis=1, keepdims=True)
    p = np.exp(logits - m, dtype=np.float32)
    p /= p.sum(axis=1, keepdims=True)
    order = np.argsort(-p, axis=1, kind="stable")
    idx = order[:, :TOPK]
    pv = np.take_along_axis(p, idx, axis=1)
    vals = (pv / pv.sum(axis=1, keepdims=True)).astype(np.float32)
    return idx, vals


def _pack_pm(arr_hc: np.ndarray) -> np.ndarray:
    """[H, C] -> partition-major [P, KH*C] (per partition: [k][c])."""
    h, c = arr_hc.shape
    return np.ascontiguousarray(
        arr_hc.reshape(h // P, P, c).transpose(1, 0, 2).reshape(P, -1)
    )


def kernel(x, Wg, bg, W1, b1, W2, b2):
    global LAST_RESULTS, LAST_CALL
    from concourse.bass_utils import run_bass_kernel_spmd

    bf16 = ml_dtypes.bfloat16
    x = np.asarray(x, np.float32)
    xf = x.reshape(-1, H)
    T = xf.shape[0]

    idx, vals = _route(xf, Wg, bg)
    counts = np.bincount(idx.ravel(), minlength=E)

    # Segments: experts by token count (desc), zero-count experts skipped.
    order = [int(e) for e in np.argsort(-counts, kind="stable") if counts[e] > 0]
    segs = tuple(int(counts[e]) for e in order)
    S = len(segs)
    Ctot = sum(segs)
    A0 = segs[0]
    nb0 = min(NB0, A0)
    blocks_by_seg, boffs = _plan(segs)

    use_bias = bool(np.any(np.asarray(b1, np.float32)))
    nc = _get_program(segs, use_bias)

    W1 = np.asarray(W1, np.float32)
    W2 = np.asarray(W2, np.float32)
    b1 = np.asarray(b1, np.float32)

    # Token ids / combine scales / packed x^T per segment (shared by cores).
    shards = []
    xparts = []   # per segment: [P, KH*A] partition-major bf16
    for si in range(S):
        e = order[si]
        sel = idx == e                  # [T, 2]; at most one True per row
        ids = np.nonzero(sel.any(axis=1))[0]
        sc = vals[sel]                  # row-major => aligned with ids
        shards.append((ids, sc))
        xparts.append(_pack_pm(xf[ids].T.astype(bf16)))

    in_maps = []
    for c in range(NC):
        pieces = []
        for si in range(S):
            e = order[si]
            w1s = W1[e][:, c * FFS:(c + 1) * FFS].astype(bf16)
            if si == 0:
                # merged head piece: per partition [k][(w1 512 | xb0)],
                # then one k-major piece per later x block
                x3 = xparts[0].reshape(P, KH, A0)
                w13 = _pack_pm(w1s).reshape(P, KH, FFS)
                pieces.append(np.concatenate([w13, x3[:, :, :nb0]], axis=2)
                              .reshape(P, -1))
                for t0, nb in blocks_by_seg[0][1:]:
                    pieces.append(np.ascontiguousarray(x3[:, :, t0:t0 + nb])
                                  .reshape(P, -1))
            else:
                # per partition: [k][w1 cols | x cols] contiguous
                w13 = _pack_pm(w1s).reshape(P, KH, FFS)
                x3 = xparts[si].reshape(P, KH, segs[si])
                pieces.append(np.concatenate([w13, x3], axis=2)
                              .reshape(P, -1))
        xwc = np.ascontiguousarray(np.concatenate(pieces, axis=1))
        w2c = np.concatenate(
            [_pack_pm(W2[order[si]][c * FFS:(c + 1) * FFS, :].astype(bf16))
             for si in range(S)],
            axis=1,
        )
        b1c = np.ascontiguousarray(np.stack(
            [b1[order[si]][c * FFS + f * P:c * FFS + (f + 1) * P]
             for si in range(S) for f in range(KFS)],
            axis=1,
        ))
        in_maps.append({"xw": xwc, "w2": np.ascontiguousarray(w2c), "b1p": b1c})

    LAST_CALL = (nc, in_maps)
    LAST_RESULTS = run_bass_kernel_spmd(nc, in_maps, list(range(NC)),
                                        trace=TRACE)

    # Sum partials across cores in the packed layout, then unpack.
    ysum2 = np.zeros((P, KH * Ctot), np.float32)
    for c in range(NC):
        ysum2 += LAST_RESULTS.results[c]["y"].astype(np.float32)
    ysum = np.empty((H, Ctot), np.float32)
    soff = 0
    for si in range(S):
        for bi, (t0, nb) in enumerate(blocks_by_seg[si]):
            boff = boffs[(si, bi)]
            blk = ysum2[:, boff:boff + KH * nb].reshape(P, KH, nb)
            ysum[:, soff + t0:soff + t0 + nb] = (
                blk.transpose(1, 0, 2).reshape(H, nb))
        soff += segs[si]

    out = np.zeros((T, H), np.float32)
    c0 = 0
    for si in range(S):
        ids, sc = shards[si]
        out[ids] += ysum[:, c0:c0 + ids.size].T * sc[:, None]
        c0 += segs[si]

    b2 = np.asarray(b2, np.float32)
    out += vals[:, 0:1] * b2[idx[:, 0]] + vals[:, 1:2] * b2[idx[:, 1]]
    return out.reshape(x.shape)


# revision 30
# speedup vs baseline: 1.0300x; 1.0031x over previous
"""FF-sharded MoE FFN kernel for Trainium2 (8 NeuronCores), v2 "W8".

Strategy (pure FF-tensor-parallel, single group):
  - Host computes the gate in fp32 (softmax -> top-2 -> renormalize).
  - Every core processes ALL routed (expert, token) visits; the FFN
    hidden dim (FF=4096) is sharded 8 ways: core c holds columns
    [c*512, (c+1)*512) of every expert's W1 and the matching rows of
    W2, and computes
        Ypart = gelu(X @ W1[:, shard] + b1[shard]) @ W2[shard, :]
    for each expert segment. The host sums the 8 partials, applies the
    top-2 combine weights, and adds the b2 term.
  - Why: per-core work is exactly sum(counts)/8 * H * FFS MAC columns
    for ANY routing - zero load imbalance and zero slot padding (the
    previous expert-pairing scheme padded ~1%). HBM traffic is
    ~50MB/core (16 W + 17 x + 17 y), hidden under ~265us of matmul.

Per-core schedule (8 segments = experts, descending token count):
  Inputs are packed PARTITION-MAJOR: per SBUF partition, each DMA'd
  piece is one contiguous [k][col] run, so every transfer is 128 large
  descriptors (small strided descriptors measured as low as 37GB/s;
  large ones ~245GB/s). Queue assignment is driven by measured queue
  rates: the sync queue is the fast one, so ALL latency-critical
  input (w1|x) and output (y) traffic goes to sync in exact
  consumption order; w2 and b1 (needed one GEMM-phase later) ride the
  parallel gpsimd SW-DGE queue. Every dma_start costs ~2-6us of
  queue-side latency before data flows, so segment 0 uses ONE merged
  [w1 | x-block0] head DMA (~1.5MB, first data ~13us) plus one DMA per
  later x block; GEMM2 blocks interleave one-behind GEMM1 (lag-1) so
  the PE has ~2x compute per input byte while transfers land.
  Zero-matmul warm-up (NBMAX-wide, accumulated ahead of the first real
  PSUM group) covers the initial DMA wait; it must keep the PE busy
  CONTIGUOUSLY ~2 aligned 3.4us HAM windows or the PE stays at half
  clock (any idle gap resets the window).
  All GEMMs bf16 on the PE with fp32 PSUM accumulation; exact gelu is
  fused into the GEMM1 PSUM eviction (ScalarE) with the b1 bias; GEMM2
  evictions (VectorE) write bf16 into a per-block PACKED staging tile
  so the y output DMA is one contiguous run per partition. The last
  two blocks drain in staggered 2/4-row-chunk DMAs so only ~0.25MB of
  transfer trails the final matmul.
"""

import sys

if "/opt/trn_rl_repo" not in sys.path:
    sys.path.insert(0, "/opt/trn_rl_repo")

import numpy as np
import ml_dtypes

H = 1024          # hidden size
E = 8             # experts
TOPK = 2
FF = 4 * H        # expert hidden dim
P = 128           # SBUF partitions
NC = 8            # cores == FF shards
FFS = FF // NC    # per-core FF shard (512)
KH = H // P       # 8  contraction chunks for GEMM1
KFS = FFS // P    # 4  contraction chunks for GEMM2 (shard)
NB0 = 256         # segment-0 head block width (two of them)

_prog_cache: dict[tuple, object] = {}
LAST_RESULTS = None  # BassKernelResults of the most recent run (for test harness)
TRACE = False        # test harness can set kernel.TRACE = True for profiling
ACT_OVERRIDE = None  # sim-only: CoreSim lacks Gelu; tests may set e.g. "Relu"
LAST_CALL = None     # (nc, in_maps) of the most recent run, for re-runs
WARM_N = 26          # HAM/pstate pre-warm zero-matmuls at kernel start.
# Zero matmuls are NBMAX wide; they accumulate into the first PSUM group
# ahead of the real contraction. They must keep the PE busy CONTIGUOUSLY
# until the merged head DMA lands (~16us): HAM only flips to full clock
# after ~2 aligned 3.4us windows of uninterrupted busy.
# (phase, bi, group) -> zero MMs prepended to that group (stall bridging).
BRIDGE = {}


def _seg_blocks(A: int, first: int | None = None):
    """Split A token columns into near-equal blocks <= 512.

    first: width of the first TWO blocks (segment 0 only): small head
    blocks let the first GEMMs start as soon as ~0.8MB of input has
    landed. Avoid blocks < ~230: below that LDWEIGHTS (~114ns) stops
    hiding behind the matmul stream.
    """
    blocks = []
    t = 0
    if first is not None:
        nb = min(first, A)
        blocks.append((t, nb))
        t = nb
        A -= nb
    if A > 0:
        nblk = -(-A // 512)
        base = A // nblk
        rem = A % nblk
        for i in range(nblk):
            nb = base + (1 if i < rem else 0)
            blocks.append((t, nb))
            t += nb
    return blocks


def _plan(segs: tuple[int, ...]):
    """Shared host/kernel plan: per-segment blocks + packed-y offsets.

    Returns (blocks_by_seg, boffs) where boffs[(si, bi)] is the element
    offset (per partition) of that block's [ht][t]-contiguous span in
    the packed y output.
    """
    nb0 = min(NB0, segs[0])
    blocks_by_seg = [
        _seg_blocks(A, first=nb0 if si == 0 else None)
        for si, A in enumerate(segs)
    ]
    boffs = {}
    off = 0
    for si, blocks in enumerate(blocks_by_seg):
        for bi, (t0, nb) in enumerate(blocks):
            boffs[(si, bi)] = off
            off += KH * nb
    return blocks_by_seg, boffs


def _build_program(segs: tuple[int, ...], use_bias: bool = True):
    """Build + compile the per-core SPMD Bass program.

    segs: token count per segment, descending (exact per-expert counts;
    identical on all cores).

    DRAM I/O (S = len(segs), Ctot = sum(segs)):
      xw  [P, 8*(S*FFS + Ctot)] bf16  partition-major packed inputs:
          per partition, per piece: [k][cols] contiguous (segment 0 is
          stored as separate pieces: w1c0 | xblk0 | w1c1 | w1c2.. |
          xblk1 | xblk2 ..)
      w2  [P, S*KFS*H] bf16  partition-major W2 shards
      b1p [P, S*KFS]  f32   b1 shard, col si*KFS+f = b1[f*128:(f+1)*128]
      y   [P, KH*Ctot] bf16 partial YT, packed per block: each block's
          span is [ht][t] contiguous per partition (host sums cores,
          then unpacks)
    """
    from contextlib import ExitStack

    from concourse import bacc
    import concourse.mybir as mybir
    import concourse.tile as tile

    dt = mybir.dt
    S = len(segs)
    Ctot = sum(segs)
    A0 = segs[0]
    nb0 = min(NB0, A0)
    blocks_by_seg, boffs = _plan(segs)
    NBMAX = max(nb for blocks in blocks_by_seg for _, nb in blocks)

    nc = bacc.Bacc(None, target_bir_lowering=False, debug=False)

    xw = nc.dram_tensor("xw", [P, KH * (S * FFS + Ctot)], dt.bfloat16,
                        kind="ExternalInput")
    w2 = nc.dram_tensor("w2", [P, S * KFS * H], dt.bfloat16,
                        kind="ExternalInput")
    b1p = nc.dram_tensor("b1p", [P, S * KFS], dt.float32, kind="ExternalInput")
    y = nc.dram_tensor("y", [P, KH * Ctot], dt.bfloat16, kind="ExternalOutput")

    # xw element offset (per partition) of each segment's packed block;
    # segment 0 occupies [0, KH*(FFS+A0)) split into its pieces.
    seg_off = [0]
    for A in segs:
        seg_off.append(seg_off[-1] + KH * (FFS + A))

    def xw_src(elem_off: int, ncols: int):
        """2D contiguous per-partition run of xw (128 big descriptors --
        3D APs here cost ~2.6us of DGE descriptor-generation per DMA)."""
        a = elem_off
        return xw[:, a:a + KH * ncols]

    with ExitStack() as ctx:
        tc = ctx.enter_context(tile.TileContext(nc))
        xwpool = ctx.enter_context(tc.tile_pool(name="xwpool", bufs=2))
        w2pool = ctx.enter_context(tc.tile_pool(name="w2pool", bufs=2))
        bpool = ctx.enter_context(tc.tile_pool(name="bpool", bufs=1))
        hpool = ctx.enter_context(tc.tile_pool(name="hpool", bufs=2))
        psA = ctx.enter_context(tc.tile_pool(name="psA", bufs=4, space="PSUM"))
        psB = ctx.enter_context(tc.tile_pool(name="psB", bufs=4, space="PSUM"))
        opool = ctx.enter_context(tc.tile_pool(name="opool", bufs=3))

        act = getattr(mybir.ActivationFunctionType, ACT_OVERRIDE or "Gelu")
        tiles = {}

        b1t = bpool.tile([P, S * KFS], dt.float32, tag="b1t", name="b1t")

        # --- segment 0: a merged [w1 | x-block0] head tile (ONE dma --
        # each dma instruction costs ~2us of queue latency) + one tile
        # per later x block, all on sync in consumption order; w2/b1 on
        # the parallel gpsimd queue ---
        blocks0 = blocks_by_seg[0]
        head0 = bpool.tile([P, KH * (FFS + nb0)], dt.bfloat16, tag="head0",
                           name="head0")
        xts = [None] + [bpool.tile([P, KH * nb], dt.bfloat16, tag=f"xt{bi}",
                                   name=f"xt{bi}")
                        for bi, (t0, nb) in enumerate(blocks0[1:], 1)]

        def emit_seg0():
            nc.sync.dma_start(out=head0[:, :], in_=xw_src(0, FFS + nb0))
            o = KH * (FFS + nb0)
            emit_w2(0)
            if use_bias:
                nc.gpsimd.dma_start(out=b1t[:], in_=b1p[:, :])
            for bi in range(1, len(blocks0)):
                nb = blocks0[bi][1]
                nc.sync.dma_start(out=xts[bi][:, :], in_=xw_src(o, nb))
                o += KH * nb

        def emit_w2(si):
            w2t = w2pool.tile([P, KFS * H], dt.bfloat16, tag="w2t",
                              name=f"w2t{si}")
            tiles[("w2", si)] = w2t
            nc.gpsimd.dma_start(
                out=w2t[:, :], in_=w2[:, si * KFS * H:(si + 1) * KFS * H])

        def lhsT2(si, w2t, k, ht):
            """GEMM2 stationary operand: w2 ht-chunk (128 cols)."""
            a = k * H + ht * P
            return w2t[:, a:a + P]

        def emit_seg(si):
            A = segs[si]
            ct = xwpool.tile([P, KH * (FFS + A)], dt.bfloat16, tag="ct",
                             name=f"ct{si}")
            tiles[("ct", si)] = ct
            nc.sync.dma_start(out=ct[:, :], in_=xw_src(seg_off[si], FFS + A))
            emit_w2(si)

        def lhsT1(si, k, ff):
            """GEMM1 stationary operand: w1 ff-chunk (128 cols)."""
            if si == 0:
                a = k * (FFS + nb0) + ff * P
                return head0[:, a:a + P]
            ct = tiles[("ct", si)]
            A = segs[si]
            a = k * (FFS + A) + ff * P
            return ct[:, a:a + P]

        def rhs1(si, k, bi, t0, nb):
            """GEMM1 moving operand: x token block."""
            if si == 0:
                if bi == 0:
                    a = k * (FFS + nb0) + FFS
                    return head0[:, a:a + nb]
                nb_bi = blocks_by_seg[0][bi][1]
                a = k * nb_bi
                return xts[bi][:, a:a + nb]
            ct = tiles[("ct", si)]
            A = segs[si]
            a = k * (FFS + A) + FFS + t0
            return ct[:, a:a + nb]

        # warm-up zero tile first in the vector queue (no input deps) so
        # the PE can start ramping before any DMA lands
        warm = bpool.tile([P, NBMAX], dt.bfloat16, tag="warm", name="warm")
        nc.vector.memset(warm[:, :], 0.0)

        emit_seg0()
        if S > 1:
            emit_seg(1)

        def zero_pad(ps, nb, n):
            """n zero matmuls accumulated into a PSUM group (busy filler).

            Full NBMAX width regardless of nb: wider zeros cover more
            wall-clock per instruction; the real accumulation only reads
            ps[:, :nb].
            """
            for i in range(n):
                nc.tensor.matmul(
                    ps[:, :NBMAX],
                    lhsT=warm[:, :P],
                    rhs=warm[:, :NBMAX],
                    start=(i == 0),
                    stop=False,
                )

        def g1_block(si, hblk, bi, t0, nb):
            """GEMM1 for one token block -> hblk[:, :, t0:t0+nb]."""
            for ff in range(KFS):
                pa = psA.tile([P, NBMAX], dt.float32, tag="pa",
                              name=f"pa{si}_{bi}_{ff}")
                warm_n = 0
                if si == 0 and bi == 0 and ff == 0:
                    # Pre-warm: accumulate zero-matmuls into the first
                    # PSUM group while the first input DMAs land; also
                    # ramps the PE clock out of its cold p-state.
                    warm_n = WARM_N
                elif si == 0:
                    warm_n = BRIDGE.get(("g1", bi, ff), 0)
                zero_pad(pa, nb, warm_n)
                for k in range(KH):
                    nc.tensor.matmul(
                        pa[:, :nb],
                        lhsT=lhsT1(si, k, ff),
                        rhs=rhs1(si, k, bi, t0, nb),
                        start=(k == 0 and warm_n == 0),
                        stop=(k == KH - 1),
                    )
                if use_bias:
                    nc.scalar.activation(
                        hblk[:, ff, t0:t0 + nb],
                        pa[:, :nb],
                        act,
                        bias=b1t[:, si * KFS + ff:si * KFS + ff + 1],
                    )
                else:
                    nc.scalar.activation(
                        hblk[:, ff, t0:t0 + nb],
                        pa[:, :nb],
                        act,
                    )

        def g2_block(si, w2t, hblk, bi, t0, nb, last_seg_blocks):
            """GEMM2 for one token block -> packed y DMA."""
            boff = boffs[(si, bi)]
            # ot is PACKED at stride nb so the output DMA is one
            # contiguous [ht][t] run per partition (large descriptors)
            ot = opool.tile([P, KH * NBMAX], dt.bfloat16, tag="ot",
                            name=f"ot{si}_{bi}")
            # tail: the last two blocks drain in staggered row-chunks
            # so only a small transfer trails the final matmul
            nblk_left = last_seg_blocks - bi if si == S - 1 else 99
            if nblk_left == 1:       # final block: 2-ht then 1-ht chunks
                stagger = {1: 0, 3: 2, 5: 4, 6: 6, 7: 7}
            elif nblk_left == 2:     # second-to-last: 4-ht chunks
                stagger = {3: 0, 7: 4}
            else:
                stagger = None
            for ht in range(KH):
                pb = psB.tile([P, NBMAX], dt.float32, tag="pb",
                              name=f"pb{si}_{bi}_{ht}")
                warm_n = BRIDGE.get(("g2", bi, ht), 0) if si == 0 else 0
                zero_pad(pb, nb, warm_n)
                for k in range(KFS):
                    nc.tensor.matmul(
                        pb[:, :nb],
                        lhsT=lhsT2(si, w2t, k, ht),
                        rhs=hblk[:, k, t0:t0 + nb],
                        start=(k == 0 and warm_n == 0),
                        stop=(k == KFS - 1),
                    )
                nc.vector.tensor_copy(ot[:, ht * nb:(ht + 1) * nb],
                                      pb[:, :nb])
                if stagger is not None and ht in stagger:
                    lo = stagger[ht]
                    nc.sync.dma_start(
                        out=y[:, boff + lo * nb:boff + (ht + 1) * nb],
                        in_=ot[:, lo * nb:(ht + 1) * nb],
                    )
            if stagger is None:
                nc.sync.dma_start(
                    out=y[:, boff:boff + KH * nb],
                    in_=ot[:, :KH * nb],
                )

        for si, A in enumerate(segs):
            blocks = blocks_by_seg[si]
            nblk = len(blocks)
            hblk = hpool.tile([P, KFS, A], dt.bfloat16, tag="hblk",
                              name=f"hblk{si}")
            if si == 0:
                # Segment 0 is DMA-arrival-paced: interleave GEMM2 blocks
                # one behind GEMM1 (lag-1) so the PE has ~2x compute per
                # input byte while the head transfers land.
                w2t = tiles.pop(("w2", 0))
                for i in range(nblk + 1):
                    if i >= 1:
                        t0, nb = blocks[i - 1]
                        g2_block(0, w2t, hblk, i - 1, t0, nb, nblk)
                    if i < nblk:
                        t0, nb = blocks[i]
                        g1_block(0, hblk, i, t0, nb)
            else:
                for bi, (t0, nb) in enumerate(blocks):
                    g1_block(si, hblk, bi, t0, nb)
                    if bi == 0 and si + 1 < S:
                        # Prefetch segment si+1 while the rest of this
                        # segment computes (~28us of cover for ~4MB).
                        emit_seg(si + 1)
                w2t = tiles.pop(("w2", si))
                for bi, (t0, nb) in enumerate(blocks):
                    g2_block(si, w2t, hblk, bi, t0, nb, nblk)

    nc.compile()
    return nc


def _get_program(segs: tuple[int, ...], use_bias: bool = True):
    key = (segs, use_bias)
    if key not in _prog_cache:
        _prog_cache[key] = _build_program(segs, use_bias)
    return _prog_cache[key]


def _route(xf: np.ndarray, Wg: np.ndarray, bg: np.ndarray):
    """fp32 gate: softmax -> top-2 (stable order, matches jax top_k) -> renorm."""
    logits = xf @ np.asarray(Wg, np.float32) + np.asarray(bg, np.float32)
    m = logits.max(axis=1, keepdims=True)
    p = np.exp(logits - m, dtype=np.float32)
    p /= p.sum(axis=1, keepdims=True)
    order = np.argsort(-p, axis=1, kind="stable")
    idx = order[:, :TOPK]
    pv = np.take_along_axis(p, idx, axis=1)
    vals = (pv / pv.sum(axis=1, keepdims=True)).astype(np.float32)
    return idx, vals


def _pack_pm(arr_hc: np.ndarray) -> np.ndarray:
    """[H, C] -> partition-major [P, KH*C] (per partition: [k][c])."""
    h, c = arr_hc.shape
    return np.ascontiguousarray(
        arr_hc.reshape(h // P, P, c).transpose(1, 0, 2).reshape(P, -1)
    )


def kernel(x, Wg, bg, W1, b1, W2, b2):
    global LAST_RESULTS, LAST_CALL
    from concourse.bass_utils import run_bass_kernel_spmd

    bf16 = ml_dtypes.bfloat16
    x = np.asarray(x, np.float32)
    xf = x.reshape(-1, H)
    T = xf.shape[0]

    idx, vals = _route(xf, Wg, bg)
    counts = np.bincount(idx.ravel(), minlength=E)

    # Segments: experts by token count (desc), zero-count experts skipped.
    order = [int(e) for e in np.argsort(-counts, kind="stable") if counts[e] > 0]
    segs = tuple(int(counts[e]) for e in order)
    S = len(segs)
    Ctot = sum(segs)
    A0 = segs[0]
    nb0 = min(NB0, A0)
    blocks_by_seg, boffs = _plan(segs)

    use_bias = bool(np.any(np.asarray(b1, np.float32)))
    nc = _get_program(segs, use_bias)

    W1 = np.asarray(W1, np.float32)
    W2 = np.asarray(W2, np.float32)
    b1 = np.asarray(b1, np.float32)

    # Token ids / combine scales / packed x^T per segment (shared by cores).
    shards = []
    xparts = []   # per segment: [P, KH*A] partition-major bf16
    for si in range(S):
        e = order[si]
        sel = idx == e                  # [T, 2]; at most one True per row
        ids = np.nonzero(sel.any(axis=1))[0]
        sc = vals[sel]                  # row-major => aligned with ids
        shards.append((ids, sc))
        xparts.append(_pack_pm(xf[ids].T.astype(bf16)))

    in_maps = []
    for c in range(NC):
        pieces = []
        for si in range(S):
            e = order[si]
            w1s = W1[e][:, c * FFS:(c + 1) * FFS].astype(bf16)
            if si == 0:
                # merged head piece: per partition [k][(w1 512 | xb0)],
                # then one k-major piece per later x block
                x3 = xparts[0].reshape(P, KH, A0)
                w13 = _pack_pm(w1s).reshape(P, KH, FFS)
                pieces.append(np.concatenate([w13, x3[:, :, :nb0]], axis=2)
                              .reshape(P, -1))
                for t0, nb in blocks_by_seg[0][1:]:
                    pieces.append(np.ascontiguousarray(x3[:, :, t0:t0 + nb])
                                  .reshape(P, -1))
            else:
                # per partition: [k][w1 cols | x cols] contiguous
                w13 = _pack_pm(w1s).reshape(P, KH, FFS)
                x3 = xparts[si].reshape(P, KH, segs[si])
                pieces.append(np.concatenate([w13, x3], axis=2)
                              .reshape(P, -1))
        xwc = np.ascontiguousarray(np.concatenate(pieces, axis=1))
        w2c = np.concatenate(
            [_pack_pm(W2[order[si]][c * FFS:(c + 1) * FFS, :].astype(bf16))
             for si in range(S)],
            axis=1,
        )
        b1c = np.ascontiguousarray(np.stack(
            [b1[order[si]][c * FFS + f * P:c * FFS + (f + 1) * P]
             for si in range(S) for f in range(KFS)],
            axis=1,
        ))
        in_maps.append({"xw": xwc, "w2": np.ascontiguousarray(w2c), "b1p": b1c})

    LAST_CALL = (nc, in_maps)
    LAST_RESULTS = run_bass_kernel_spmd(nc, in_maps, list(range(NC)),
                                        trace=TRACE)

    # Sum partials across cores in the packed layout, then unpack.
    ysum2 = np.zeros((P, KH * Ctot), np.float32)
    for c in range(NC):
        ysum2 += LAST_RESULTS.results[c]["y"].astype(np.float32)
    ysum = np.empty((H, Ctot), np.float32)
    soff = 0
    for si in range(S):
        for bi, (t0, nb) in enumerate(blocks_by_seg[si]):
            boff = boffs[(si, bi)]
            blk = ysum2[:, boff:boff + KH * nb].reshape(P, KH, nb)
            ysum[:, soff + t0:soff + t0 + nb] = (
                blk.transpose(1, 0, 2).reshape(H, nb))
        soff += segs[si]

    out = np.zeros((T, H), np.float32)
    c0 = 0
    for si in range(S):
        ids, sc = shards[si]
        out[ids] += ysum[:, c0:c0 + ids.size].T * sc[:, None]
        c0 += segs[si]

    b2 = np.asarray(b2, np.float32)
    out += vals[:, 0:1] * b2[idx[:, 0]] + vals[:, 1:2] * b2[idx[:, 1]]
    return out.reshape(x.shape)


# revision 32
# speedup vs baseline: 1.0362x; 1.0060x over previous
"""FF-sharded MoE FFN kernel for Trainium2 (8 NeuronCores), v2 "W8".

Strategy (pure FF-tensor-parallel, single group):
  - Host computes the gate in fp32 (softmax -> top-2 -> renormalize).
  - Every core processes ALL routed (expert, token) visits; the FFN
    hidden dim (FF=4096) is sharded 8 ways: core c holds columns
    [c*512, (c+1)*512) of every expert's W1 and the matching rows of
    W2, and computes
        Ypart = gelu(X @ W1[:, shard] + b1[shard]) @ W2[shard, :]
    for each expert segment. The host sums the 8 partials, applies the
    top-2 combine weights, and adds the b2 term.
  - Why: per-core work is exactly sum(counts)/8 * H * FFS MAC columns
    for ANY routing - zero load imbalance and zero slot padding (the
    previous expert-pairing scheme padded ~1%). HBM traffic is
    ~50MB/core (16 W + 17 x + 17 y), hidden under ~265us of matmul.

Per-core schedule (8 segments = experts, descending token count):
  Inputs are packed PARTITION-MAJOR: per SBUF partition, each DMA'd
  piece is one contiguous [k][col] run, so every transfer is 128 large
  descriptors (small strided descriptors measured as low as 37GB/s;
  large ones ~245GB/s). Queue assignment is driven by measured queue
  rates: the sync queue is the fast one, so ALL latency-critical
  input (w1|x) and output (y) traffic goes to sync in exact
  consumption order; w2 and b1 (needed one GEMM-phase later) ride the
  parallel gpsimd SW-DGE queue. Every dma_start costs ~2-6us of
  queue-side latency before data flows, so segment 0 uses ONE merged
  [w1 | x-block0] head DMA (~1.5MB, first data ~13us) plus one DMA per
  later x block; GEMM2 blocks interleave one-behind GEMM1 (lag-1) so
  the PE has ~2x compute per input byte while transfers land.
  Zero-matmul warm-up (NBMAX-wide, accumulated ahead of the first real
  PSUM group) covers the initial DMA wait; it must keep the PE busy
  CONTIGUOUSLY ~2 aligned 3.4us HAM windows or the PE stays at half
  clock (any idle gap resets the window).
  All GEMMs bf16 on the PE with fp32 PSUM accumulation; exact gelu is
  fused into the GEMM1 PSUM eviction (ScalarE) with the b1 bias; GEMM2
  evictions (VectorE) write bf16 into a per-block PACKED staging tile
  so the y output DMA is one contiguous run per partition. The last
  two blocks drain in staggered 2/4-row-chunk DMAs so only ~0.25MB of
  transfer trails the final matmul.
"""

import sys

if "/opt/trn_rl_repo" not in sys.path:
    sys.path.insert(0, "/opt/trn_rl_repo")

import numpy as np
import ml_dtypes

H = 1024          # hidden size
E = 8             # experts
TOPK = 2
FF = 4 * H        # expert hidden dim
P = 128           # SBUF partitions
NC = 8            # cores == FF shards
FFS = FF // NC    # per-core FF shard (512)
KH = H // P       # 8  contraction chunks for GEMM1
KFS = FFS // P    # 4  contraction chunks for GEMM2 (shard)
NB0 = 256         # segment-0 head block width (two of them)

_prog_cache: dict[tuple, object] = {}
LAST_RESULTS = None  # BassKernelResults of the most recent run (for test harness)
TRACE = False        # test harness can set kernel.TRACE = True for profiling
ACT_OVERRIDE = None  # sim-only: CoreSim lacks Gelu; tests may set e.g. "Relu"
LAST_CALL = None     # (nc, in_maps) of the most recent run, for re-runs
WARM_N = 16          # HAM/pstate pre-warm zero-matmuls at kernel start.
# Zero matmuls are NBMAX wide; they accumulate into the first PSUM group
# ahead of the real contraction. They must keep the PE busy CONTIGUOUSLY
# until the merged head DMA lands (~16us): HAM only flips to full clock
# after ~2 aligned 3.4us windows of uninterrupted busy.
# (phase, bi, group) -> zero MMs prepended to that group (stall bridging).
BRIDGE = {}
# Segment 0's GEMM1 inputs (w1 + x) ship as fp8 e3m4 packed BYTE-WISE
# inside the bf16 xw tensor (marshals byte-exact); SBUF APs bitcast to
# fp8 at the matmuls. Halves the critical head DMA. Values pre-scaled
# into e3m4 range; the product scale is divided out by the gelu
# eviction's input scale. Odd block widths pad per-k stride to even
# (the matmul computes one ignored zero column).
SW0 = 16.0           # W1 scale (std 0.02 -> 0.32)
SX0 = 2.0            # x scale (std 1 -> 2)


def _seg_blocks(A: int, first: int | None = None):
    """Split A token columns into near-equal blocks <= 512.

    first: width of the first TWO blocks (segment 0 only): small head
    blocks let the first GEMMs start as soon as ~0.8MB of input has
    landed. Avoid blocks < ~230: below that LDWEIGHTS (~114ns) stops
    hiding behind the matmul stream.
    """
    blocks = []
    t = 0
    if first is not None:
        nb = min(first, A)
        blocks.append((t, nb))
        t = nb
        A -= nb
    if A > 0:
        nblk = -(-A // 512)
        base = A // nblk
        rem = A % nblk
        for i in range(nblk):
            nb = base + (1 if i < rem else 0)
            blocks.append((t, nb))
            t += nb
    return blocks


def _plan(segs: tuple[int, ...]):
    """Shared host/kernel plan: per-segment blocks + packed-y offsets.

    Returns (blocks_by_seg, boffs) where boffs[(si, bi)] is the element
    offset (per partition) of that block's [ht][t]-contiguous span in
    the packed y output.
    """
    nb0 = min(NB0, segs[0])
    blocks_by_seg = [
        _seg_blocks(A, first=nb0 if si == 0 else None)
        for si, A in enumerate(segs)
    ]
    boffs = {}
    off = 0
    for si, blocks in enumerate(blocks_by_seg):
        for bi, (t0, nb) in enumerate(blocks):
            boffs[(si, bi)] = off
            off += KH * nb
    return blocks_by_seg, boffs


def _build_program(segs: tuple[int, ...], use_bias: bool = True):
    """Build + compile the per-core SPMD Bass program.

    segs: token count per segment, descending (exact per-expert counts;
    identical on all cores).

    DRAM I/O (S = len(segs), Ctot = sum(segs)):
      xw  [P, 8*(S*FFS + Ctot)] bf16  partition-major packed inputs:
          per partition, per piece: [k][cols] contiguous (segment 0 is
          stored as separate pieces: w1c0 | xblk0 | w1c1 | w1c2.. |
          xblk1 | xblk2 ..)
      w2  [P, S*KFS*H] bf16  partition-major W2 shards
      b1p [P, S*KFS]  f32   b1 shard, col si*KFS+f = b1[f*128:(f+1)*128]
      y   [P, KH*Ctot] bf16 partial YT, packed per block: each block's
          span is [ht][t] contiguous per partition (host sums cores,
          then unpacks)
    """
    from contextlib import ExitStack

    from concourse import bacc
    import concourse.mybir as mybir
    import concourse.tile as tile

    dt = mybir.dt
    S = len(segs)
    Ctot = sum(segs)
    A0 = segs[0]
    nb0 = min(NB0, A0)
    blocks_by_seg, boffs = _plan(segs)
    NBMAX = max(nb for blocks in blocks_by_seg for _, nb in blocks)

    nc = bacc.Bacc(None, target_bir_lowering=False, debug=False)

    _b0 = blocks_by_seg[0]
    _seg0w = KH * (FFS + _b0[0][1]) // 2 + sum(
        KH * (nb + (nb & 1)) // 2 for _, nb in _b0[1:])
    xw = nc.dram_tensor(
        "xw", [P, _seg0w + KH * ((S - 1) * FFS + Ctot - A0)],
        dt.bfloat16, kind="ExternalInput")
    w2 = nc.dram_tensor("w2", [P, S * KFS * H], dt.bfloat16,
                        kind="ExternalInput")
    b1p = nc.dram_tensor("b1p", [P, S * KFS], dt.float32, kind="ExternalInput")
    y = nc.dram_tensor("y", [P, KH * Ctot], dt.bfloat16, kind="ExternalOutput")

    # xw element offset (per partition) of each segment's packed block;
    # segment 0 occupies [0, KH*(FFS+A0)) split into its pieces.
    blocks0_ = blocks_by_seg[0]
    seg0_cols = KH * (FFS + blocks0_[0][1]) // 2 + sum(
        KH * (nb + (nb & 1)) // 2 for _, nb in blocks0_[1:])
    seg_off = [None, seg0_cols]
    for A in segs[1:]:
        seg_off.append(seg_off[-1] + KH * (FFS + A))

    def xw_src(elem_off: int, ncols: int):
        """2D contiguous per-partition run of xw (128 big descriptors --
        3D APs here cost ~2.6us of DGE descriptor-generation per DMA)."""
        a = elem_off
        return xw[:, a:a + KH * ncols]

    with ExitStack() as ctx:
        tc = ctx.enter_context(tile.TileContext(nc))
        xwpool = ctx.enter_context(tc.tile_pool(name="xwpool", bufs=2))
        w2pool = ctx.enter_context(tc.tile_pool(name="w2pool", bufs=2))
        bpool = ctx.enter_context(tc.tile_pool(name="bpool", bufs=1))
        hpool = ctx.enter_context(tc.tile_pool(name="hpool", bufs=2))
        psA = ctx.enter_context(tc.tile_pool(name="psA", bufs=4, space="PSUM"))
        psB = ctx.enter_context(tc.tile_pool(name="psB", bufs=4, space="PSUM"))
        opool = ctx.enter_context(tc.tile_pool(name="opool", bufs=3))

        act = getattr(mybir.ActivationFunctionType, ACT_OVERRIDE or "Gelu")
        tiles = {}

        b1t = bpool.tile([P, S * KFS], dt.float32, tag="b1t", name="b1t")

        # --- segment 0: a merged [w1 | x-block0] head tile (ONE dma --
        # each dma instruction costs ~2us of queue latency) + one tile
        # per later x block, all on sync in consumption order; w2/b1 on
        # the parallel gpsimd queue ---
        blocks0 = blocks_by_seg[0]
        HW0 = KH * (FFS + nb0) // 2          # head0 width in bf16 cols
        head0 = bpool.tile([P, HW0], dt.bfloat16, tag="head0", name="head0")

        def _nbp(nb):
            return nb + (nb & 1)             # even per-k stride for fp8

        xts = [None] + [bpool.tile([P, KH * _nbp(nb) // 2], dt.bfloat16,
                                   tag=f"xt{bi}", name=f"xt{bi}")
                        for bi, (t0, nb) in enumerate(blocks0[1:], 1)]

        def emit_seg0():
            nc.sync.dma_start(out=head0[:, :], in_=xw[:, :HW0])
            o = HW0
            emit_w2(0)
            if use_bias:
                nc.gpsimd.dma_start(out=b1t[:], in_=b1p[:, :])
            for bi in range(1, len(blocks0)):
                w = KH * _nbp(blocks0[bi][1]) // 2
                nc.sync.dma_start(out=xts[bi][:, :], in_=xw[:, o:o + w])
                o += w

        def emit_w2(si):
            w2t = w2pool.tile([P, KFS * H], dt.bfloat16, tag="w2t",
                              name=f"w2t{si}")
            tiles[("w2", si)] = w2t
            nc.gpsimd.dma_start(
                out=w2t[:, :], in_=w2[:, si * KFS * H:(si + 1) * KFS * H])

        def lhsT2(si, w2t, k, ht):
            """GEMM2 stationary operand: w2 ht-chunk (128 cols)."""
            a = k * H + ht * P
            return w2t[:, a:a + P]

        def emit_seg(si):
            A = segs[si]
            ct = xwpool.tile([P, KH * (FFS + A)], dt.bfloat16, tag="ct",
                             name=f"ct{si}")
            tiles[("ct", si)] = ct
            nc.sync.dma_start(out=ct[:, :], in_=xw_src(seg_off[si], FFS + A))
            emit_w2(si)

        def lhsT1(si, k, ff):
            """GEMM1 stationary operand: w1 ff-chunk (128 cols)."""
            if si == 0:
                a = (k * (FFS + nb0) + ff * P) // 2
                return head0[:, a:a + P // 2].bitcast(dt.float8e3)
            ct = tiles[("ct", si)]
            A = segs[si]
            a = k * (FFS + A) + ff * P
            return ct[:, a:a + P]

        def rhs1(si, k, bi, t0, nb):
            """GEMM1 moving operand: x token block (seg0: fp8, width
            padded to even -- the extra zero column lands in PSUM and is
            ignored by the eviction)."""
            if si == 0:
                if bi == 0:
                    a = (k * (FFS + nb0) + FFS) // 2
                    return head0[:, a:a + nb // 2].bitcast(dt.float8e3)
                nbp = _nbp(blocks_by_seg[0][bi][1])
                a = k * nbp // 2
                return xts[bi][:, a:a + nbp // 2].bitcast(dt.float8e3)
            ct = tiles[("ct", si)]
            A = segs[si]
            a = k * (FFS + A) + FFS + t0
            return ct[:, a:a + nb]

        # warm-up zero tile first in the vector queue (no input deps) so
        # the PE can start ramping before any DMA lands
        warm = bpool.tile([P, NBMAX], dt.bfloat16, tag="warm", name="warm")
        nc.vector.memset(warm[:, :], 0.0)

        emit_seg0()
        if S > 1:
            emit_seg(1)

        def zero_pad(ps, nb, n):
            """n zero matmuls accumulated into a PSUM group (busy filler).

            Full NBMAX width regardless of nb: wider zeros cover more
            wall-clock per instruction; the real accumulation only reads
            ps[:, :nb].
            """
            for i in range(n):
                nc.tensor.matmul(
                    ps[:, :NBMAX],
                    lhsT=warm[:, :P],
                    rhs=warm[:, :NBMAX],
                    start=(i == 0),
                    stop=False,
                )

        def g1_block(si, hblk, bi, t0, nb):
            """GEMM1 for one token block -> hblk[:, :, t0:t0+nb]."""
            for ff in range(KFS):
                pa = psA.tile([P, NBMAX], dt.float32, tag="pa",
                              name=f"pa{si}_{bi}_{ff}")
                warm_n = 0
                if si == 0 and bi == 0 and ff == 0:
                    # Pre-warm: accumulate zero-matmuls into the first
                    # PSUM group while the first input DMAs land; also
                    # ramps the PE clock out of its cold p-state.
                    warm_n = WARM_N
                elif si == 0:
                    warm_n = BRIDGE.get(("g1", bi, ff), 0)
                zero_pad(pa, nb, warm_n)
                nbw = _nbp(nb) if si == 0 else nb
                for k in range(KH):
                    nc.tensor.matmul(
                        pa[:, :nbw],
                        lhsT=lhsT1(si, k, ff),
                        rhs=rhs1(si, k, bi, t0, nb),
                        start=(k == 0 and warm_n == 0),
                        stop=(k == KH - 1),
                    )
                dscale = 1.0 / (SW0 * SX0) if si == 0 else 1.0
                if use_bias:
                    nc.scalar.activation(
                        hblk[:, ff, t0:t0 + nb],
                        pa[:, :nb],
                        act,
                        bias=b1t[:, si * KFS + ff:si * KFS + ff + 1],
                        scale=dscale,
                    )
                else:
                    nc.scalar.activation(
                        hblk[:, ff, t0:t0 + nb],
                        pa[:, :nb],
                        act,
                        scale=dscale,
                    )

        def g2_block(si, w2t, hblk, bi, t0, nb, last_seg_blocks):
            """GEMM2 for one token block -> packed y DMA."""
            boff = boffs[(si, bi)]
            # ot is PACKED at stride nb so the output DMA is one
            # contiguous [ht][t] run per partition (large descriptors)
            ot = opool.tile([P, KH * NBMAX], dt.bfloat16, tag="ot",
                            name=f"ot{si}_{bi}")
            # tail: the last two blocks drain in staggered row-chunks
            # so only a small transfer trails the final matmul
            nblk_left = last_seg_blocks - bi if si == S - 1 else 99
            if nblk_left == 1:       # final block: 2-ht then 1-ht chunks
                stagger = {1: 0, 3: 2, 5: 4, 6: 6, 7: 7}
            elif nblk_left == 2:     # second-to-last: 4-ht chunks
                stagger = {3: 0, 7: 4}
            else:
                stagger = None
            for ht in range(KH):
                pb = psB.tile([P, NBMAX], dt.float32, tag="pb",
                              name=f"pb{si}_{bi}_{ht}")
                warm_n = BRIDGE.get(("g2", bi, ht), 0) if si == 0 else 0
                zero_pad(pb, nb, warm_n)
                for k in range(KFS):
                    nc.tensor.matmul(
                        pb[:, :nb],
                        lhsT=lhsT2(si, w2t, k, ht),
                        rhs=hblk[:, k, t0:t0 + nb],
                        start=(k == 0 and warm_n == 0),
                        stop=(k == KFS - 1),
                    )
                nc.vector.tensor_copy(ot[:, ht * nb:(ht + 1) * nb],
                                      pb[:, :nb])
                if stagger is not None and ht in stagger:
                    lo = stagger[ht]
                    nc.sync.dma_start(
                        out=y[:, boff + lo * nb:boff + (ht + 1) * nb],
                        in_=ot[:, lo * nb:(ht + 1) * nb],
                    )
            if stagger is None:
                nc.sync.dma_start(
                    out=y[:, boff:boff + KH * nb],
                    in_=ot[:, :KH * nb],
                )

        for si, A in enumerate(segs):
            blocks = blocks_by_seg[si]
            nblk = len(blocks)
            hblk = hpool.tile([P, KFS, A], dt.bfloat16, tag="hblk",
                              name=f"hblk{si}")
            if si == 0:
                # Segment 0 is DMA-arrival-paced: interleave GEMM2 blocks
                # one behind GEMM1 (lag-1) so the PE has ~2x compute per
                # input byte while the head transfers land.
                w2t = tiles.pop(("w2", 0))
                for i in range(nblk + 2):
                    if i >= 2:
                        t0, nb = blocks[i - 2]
                        g2_block(0, w2t, hblk, i - 2, t0, nb, nblk)
                    if i < nblk:
                        t0, nb = blocks[i]
                        g1_block(0, hblk, i, t0, nb)
            else:
                for bi, (t0, nb) in enumerate(blocks):
                    g1_block(si, hblk, bi, t0, nb)
                    if bi == 0 and si + 1 < S:
                        # Prefetch segment si+1 while the rest of this
                        # segment computes (~28us of cover for ~4MB).
                        emit_seg(si + 1)
                w2t = tiles.pop(("w2", si))
                for bi, (t0, nb) in enumerate(blocks):
                    g2_block(si, w2t, hblk, bi, t0, nb, nblk)

    nc.compile()
    return nc


def _get_program(segs: tuple[int, ...], use_bias: bool = True):
    key = (segs, use_bias)
    if key not in _prog_cache:
        _prog_cache[key] = _build_program(segs, use_bias)
    return _prog_cache[key]


def _route(xf: np.ndarray, Wg: np.ndarray, bg: np.ndarray):
    """fp32 gate: softmax -> top-2 (stable order, matches jax top_k) -> renorm."""
    logits = xf @ np.asarray(Wg, np.float32) + np.asarray(bg, np.float32)
    m = logits.max(axis=1, keepdims=True)
    p = np.exp(logits - m, dtype=np.float32)
    p /= p.sum(axis=1, keepdims=True)
    order = np.argsort(-p, axis=1, kind="stable")
    idx = order[:, :TOPK]
    pv = np.take_along_axis(p, idx, axis=1)
    vals = (pv / pv.sum(axis=1, keepdims=True)).astype(np.float32)
    return idx, vals


def _pack_pm(arr_hc: np.ndarray) -> np.ndarray:
    """[H, C] -> partition-major [P, KH*C] (per partition: [k][c])."""
    h, c = arr_hc.shape
    return np.ascontiguousarray(
        arr_hc.reshape(h // P, P, c).transpose(1, 0, 2).reshape(P, -1)
    )


def kernel(x, Wg, bg, W1, b1, W2, b2):
    global LAST_RESULTS, LAST_CALL
    from concourse.bass_utils import run_bass_kernel_spmd

    bf16 = ml_dtypes.bfloat16
    x = np.asarray(x, np.float32)
    xf = x.reshape(-1, H)
    T = xf.shape[0]

    idx, vals = _route(xf, Wg, bg)
    counts = np.bincount(idx.ravel(), minlength=E)

    # Segments: experts by token count (desc), zero-count experts skipped.
    order = [int(e) for e in np.argsort(-counts, kind="stable") if counts[e] > 0]
    segs = tuple(int(counts[e]) for e in order)
    S = len(segs)
    Ctot = sum(segs)
    A0 = segs[0]
    nb0 = min(NB0, A0)
    blocks_by_seg, boffs = _plan(segs)

    use_bias = bool(np.any(np.asarray(b1, np.float32)))
    nc = _get_program(segs, use_bias)

    W1 = np.asarray(W1, np.float32)
    W2 = np.asarray(W2, np.float32)
    b1 = np.asarray(b1, np.float32)

    # Token ids / combine scales / packed x^T per segment (shared by cores).
    shards = []
    e3 = ml_dtypes.float8_e3m4
    xparts = []   # per segment: [P, KH*A] partition-major (seg0: fp8)
    for si in range(S):
        e = order[si]
        sel = idx == e                  # [T, 2]; at most one True per row
        ids = np.nonzero(sel.any(axis=1))[0]
        sc = vals[sel]                  # row-major => aligned with ids
        shards.append((ids, sc))
        if si == 0:
            xq = np.clip(xf[ids].T * SX0, -15.5, 15.5).astype(e3)
            xparts.append(_pack_pm(xq))
        else:
            xparts.append(_pack_pm(xf[ids].T.astype(bf16)))

    in_maps = []
    for c in range(NC):
        pieces = []
        for si in range(S):
            e = order[si]
            if si == 0:
                # fp8 pieces, byte-packed into the bf16 tensor: merged
                # [k][(w1*SW0 | xb0*SX0)] head, then per-block x pieces
                # with per-k stride padded to even
                w1q = np.clip(
                    W1[e][:, c * FFS:(c + 1) * FFS] * SW0,
                    -15.5, 15.5).astype(e3)
                x3 = xparts[0].reshape(P, KH, A0)
                w13 = _pack_pm(w1q).reshape(P, KH, FFS)
                hp = np.concatenate([w13, x3[:, :, :nb0]], axis=2)
                pieces.append(np.ascontiguousarray(hp).reshape(P, -1)
                              .view(bf16))
                for t0, nb in blocks_by_seg[0][1:]:
                    xp = x3[:, :, t0:t0 + nb]
                    if nb & 1:
                        pad = np.zeros((P, KH, 1), e3)
                        xp = np.concatenate([xp, pad], axis=2)
                    pieces.append(np.ascontiguousarray(xp).reshape(P, -1)
                                  .view(bf16))
            else:
                # per partition: [k][w1 cols | x cols] contiguous
                w1s = W1[e][:, c * FFS:(c + 1) * FFS].astype(bf16)
                w13 = _pack_pm(w1s).reshape(P, KH, FFS)
                x3 = xparts[si].reshape(P, KH, segs[si])
                pieces.append(np.concatenate([w13, x3], axis=2)
                              .reshape(P, -1))
        xwc = np.ascontiguousarray(np.concatenate(pieces, axis=1))
        w2c = np.concatenate(
            [_pack_pm(W2[order[si]][c * FFS:(c + 1) * FFS, :].astype(bf16))
             for si in range(S)],
            axis=1,
        )
        b1c = np.ascontiguousarray(np.stack(
            [b1[order[si]][c * FFS + f * P:c * FFS + (f + 1) * P]
             for si in range(S) for f in range(KFS)],
            axis=1,
        ))
        in_maps.append({"xw": xwc, "w2": np.ascontiguousarray(w2c), "b1p": b1c})

    LAST_CALL = (nc, in_maps)
    LAST_RESULTS = run_bass_kernel_spmd(nc, in_maps, list(range(NC)),
                                        trace=TRACE)

    # Sum partials across cores in the packed layout, then unpack.
    ysum2 = np.zeros((P, KH * Ctot), np.float32)
    for c in range(NC):
        ysum2 += LAST_RESULTS.results[c]["y"].astype(np.float32)
    ysum = np.empty((H, Ctot), np.float32)
    soff = 0
    for si in range(S):
        for bi, (t0, nb) in enumerate(blocks_by_seg[si]):
            boff = boffs[(si, bi)]
            blk = ysum2[:, boff:boff + KH * nb].reshape(P, KH, nb)
            ysum[:, soff + t0:soff + t0 + nb] = (
                blk.transpose(1, 0, 2).reshape(H, nb))
        soff += segs[si]

    out = np.zeros((T, H), np.float32)
    c0 = 0
    for si in range(S):
        ids, sc = shards[si]
        out[ids] += ysum[:, c0:c0 + ids.size].T * sc[:, None]
        c0 += segs[si]

    b2 = np.asarray(b2, np.float32)
    out += vals[:, 0:1] * b2[idx[:, 0]] + vals[:, 1:2] * b2[idx[:, 1]]
    return out.reshape(x.shape)


# revision 33
# speedup vs baseline: 1.0402x; 1.0039x over previous
"""FF-sharded MoE FFN kernel for Trainium2 (8 NeuronCores), v2 "W8".

Strategy (pure FF-tensor-parallel, single group):
  - Host computes the gate in fp32 (softmax -> top-2 -> renormalize).
  - Every core processes ALL routed (expert, token) visits; the FFN
    hidden dim (FF=4096) is sharded 8 ways: core c holds columns
    [c*512, (c+1)*512) of every expert's W1 and the matching rows of
    W2, and computes
        Ypart = gelu(X @ W1[:, shard] + b1[shard]) @ W2[shard, :]
    for each expert segment. The host sums the 8 partials, applies the
    top-2 combine weights, and adds the b2 term.
  - Why: per-core work is exactly sum(counts)/8 * H * FFS MAC columns
    for ANY routing - zero load imbalance and zero slot padding (the
    previous expert-pairing scheme padded ~1%). HBM traffic is
    ~50MB/core (16 W + 17 x + 17 y), hidden under ~265us of matmul.

Per-core schedule (8 segments = experts, descending token count):
  Inputs are packed PARTITION-MAJOR: per SBUF partition, each DMA'd
  piece is one contiguous [k][col] run, so every transfer is 128 large
  descriptors (small strided descriptors measured as low as 37GB/s;
  large ones ~245GB/s). Queue assignment is driven by measured queue
  rates: the sync queue is the fast one, so ALL latency-critical
  input (w1|x) and output (y) traffic goes to sync in exact
  consumption order; w2 and b1 (needed one GEMM-phase later) ride the
  parallel gpsimd SW-DGE queue. Every dma_start costs ~2-6us of
  queue-side latency before data flows, so segment 0 uses ONE merged
  [w1 | x-block0] head DMA (~1.5MB, first data ~13us) plus one DMA per
  later x block; GEMM2 blocks interleave one-behind GEMM1 (lag-1) so
  the PE has ~2x compute per input byte while transfers land.
  Zero-matmul warm-up (NBMAX-wide, accumulated ahead of the first real
  PSUM group) covers the initial DMA wait; it must keep the PE busy
  CONTIGUOUSLY ~2 aligned 3.4us HAM windows or the PE stays at half
  clock (any idle gap resets the window).
  All GEMMs bf16 on the PE with fp32 PSUM accumulation; exact gelu is
  fused into the GEMM1 PSUM eviction (ScalarE) with the b1 bias; GEMM2
  evictions (VectorE) write bf16 into a per-block PACKED staging tile
  so the y output DMA is one contiguous run per partition. The last
  two blocks drain in staggered 2/4-row-chunk DMAs so only ~0.25MB of
  transfer trails the final matmul.
"""

import sys

if "/opt/trn_rl_repo" not in sys.path:
    sys.path.insert(0, "/opt/trn_rl_repo")

import numpy as np
import ml_dtypes

H = 1024          # hidden size
E = 8             # experts
TOPK = 2
FF = 4 * H        # expert hidden dim
P = 128           # SBUF partitions
NC = 8            # cores == FF shards
FFS = FF // NC    # per-core FF shard (512)
KH = H // P       # 8  contraction chunks for GEMM1
KFS = FFS // P    # 4  contraction chunks for GEMM2 (shard)
NB0 = 320         # segment-0 head block width

_prog_cache: dict[tuple, object] = {}
LAST_RESULTS = None  # BassKernelResults of the most recent run (for test harness)
TRACE = False        # test harness can set kernel.TRACE = True for profiling
ACT_OVERRIDE = None  # sim-only: CoreSim lacks Gelu; tests may set e.g. "Relu"
LAST_CALL = None     # (nc, in_maps) of the most recent run, for re-runs
WARM_N = 17          # HAM/pstate pre-warm zero-matmuls at kernel start.
# Zero matmuls are NBMAX wide; they accumulate into the first PSUM group
# ahead of the real contraction. They must keep the PE busy CONTIGUOUSLY
# until the merged head DMA lands (~16us): HAM only flips to full clock
# after ~2 aligned 3.4us windows of uninterrupted busy.
# (phase, bi, group) -> zero MMs prepended to that group (stall bridging).
BRIDGE = {}
# Segment 0's GEMM1 inputs (w1 + x) ship as fp8 e3m4 packed BYTE-WISE
# inside the bf16 xw tensor (marshals byte-exact); SBUF APs bitcast to
# fp8 at the matmuls. Halves the critical head DMA. Values pre-scaled
# into e3m4 range; the product scale is divided out by the gelu
# eviction's input scale. Odd block widths pad per-k stride to even
# (the matmul computes one ignored zero column).
SW0 = 16.0           # W1 scale (std 0.02 -> 0.32)
SX0 = 2.0            # x scale (std 1 -> 2)


def _seg_blocks(A: int, first: int | None = None):
    """Split A token columns into near-equal blocks <= 512.

    first: width of the first TWO blocks (segment 0 only): small head
    blocks let the first GEMMs start as soon as ~0.8MB of input has
    landed. Avoid blocks < ~230: below that LDWEIGHTS (~114ns) stops
    hiding behind the matmul stream.
    """
    blocks = []
    t = 0
    if first is not None:
        nb = min(first, A)
        blocks.append((t, nb))
        t = nb
        A -= nb
    if A > 0:
        nblk = -(-A // 512)
        base = A // nblk
        rem = A % nblk
        for i in range(nblk):
            nb = base + (1 if i < rem else 0)
            blocks.append((t, nb))
            t += nb
    return blocks


def _plan(segs: tuple[int, ...]):
    """Shared host/kernel plan: per-segment blocks + packed-y offsets.

    Returns (blocks_by_seg, boffs) where boffs[(si, bi)] is the element
    offset (per partition) of that block's [ht][t]-contiguous span in
    the packed y output.
    """
    nb0 = min(NB0, segs[0])
    blocks_by_seg = [
        _seg_blocks(A, first=nb0 if si == 0 else None)
        for si, A in enumerate(segs)
    ]
    boffs = {}
    off = 0
    for si, blocks in enumerate(blocks_by_seg):
        for bi, (t0, nb) in enumerate(blocks):
            boffs[(si, bi)] = off
            off += KH * nb
    return blocks_by_seg, boffs


def _build_program(segs: tuple[int, ...], use_bias: bool = True):
    """Build + compile the per-core SPMD Bass program.

    segs: token count per segment, descending (exact per-expert counts;
    identical on all cores).

    DRAM I/O (S = len(segs), Ctot = sum(segs)):
      xw  [P, 8*(S*FFS + Ctot)] bf16  partition-major packed inputs:
          per partition, per piece: [k][cols] contiguous (segment 0 is
          stored as separate pieces: w1c0 | xblk0 | w1c1 | w1c2.. |
          xblk1 | xblk2 ..)
      w2  [P, S*KFS*H] bf16  partition-major W2 shards
      b1p [P, S*KFS]  f32   b1 shard, col si*KFS+f = b1[f*128:(f+1)*128]
      y   [P, KH*Ctot] bf16 partial YT, packed per block: each block's
          span is [ht][t] contiguous per partition (host sums cores,
          then unpacks)
    """
    from contextlib import ExitStack

    from concourse import bacc
    import concourse.mybir as mybir
    import concourse.tile as tile

    dt = mybir.dt
    S = len(segs)
    Ctot = sum(segs)
    A0 = segs[0]
    nb0 = min(NB0, A0)
    blocks_by_seg, boffs = _plan(segs)
    NBMAX = max(nb for blocks in blocks_by_seg for _, nb in blocks)

    nc = bacc.Bacc(None, target_bir_lowering=False, debug=False)

    _b0 = blocks_by_seg[0]
    _seg0w = KH * (FFS + _b0[0][1]) // 2 + sum(
        KH * (nb + (nb & 1)) // 2 for _, nb in _b0[1:])
    xw = nc.dram_tensor(
        "xw", [P, _seg0w + KH * ((S - 1) * FFS + Ctot - A0)],
        dt.bfloat16, kind="ExternalInput")
    w2 = nc.dram_tensor("w2", [P, S * KFS * H], dt.bfloat16,
                        kind="ExternalInput")
    b1p = nc.dram_tensor("b1p", [P, S * KFS], dt.float32, kind="ExternalInput")
    y = nc.dram_tensor("y", [P, KH * Ctot], dt.bfloat16, kind="ExternalOutput")

    # xw element offset (per partition) of each segment's packed block;
    # segment 0 occupies [0, KH*(FFS+A0)) split into its pieces.
    blocks0_ = blocks_by_seg[0]
    seg0_cols = KH * (FFS + blocks0_[0][1]) // 2 + sum(
        KH * (nb + (nb & 1)) // 2 for _, nb in blocks0_[1:])
    seg_off = [None, seg0_cols]
    for A in segs[1:]:
        seg_off.append(seg_off[-1] + KH * (FFS + A))

    def xw_src(elem_off: int, ncols: int):
        """2D contiguous per-partition run of xw (128 big descriptors --
        3D APs here cost ~2.6us of DGE descriptor-generation per DMA)."""
        a = elem_off
        return xw[:, a:a + KH * ncols]

    with ExitStack() as ctx:
        tc = ctx.enter_context(tile.TileContext(nc))
        xwpool = ctx.enter_context(tc.tile_pool(name="xwpool", bufs=2))
        w2pool = ctx.enter_context(tc.tile_pool(name="w2pool", bufs=2))
        bpool = ctx.enter_context(tc.tile_pool(name="bpool", bufs=1))
        hpool = ctx.enter_context(tc.tile_pool(name="hpool", bufs=2))
        psA = ctx.enter_context(tc.tile_pool(name="psA", bufs=4, space="PSUM"))
        psB = ctx.enter_context(tc.tile_pool(name="psB", bufs=4, space="PSUM"))
        opool = ctx.enter_context(tc.tile_pool(name="opool", bufs=3))

        act = getattr(mybir.ActivationFunctionType, ACT_OVERRIDE or "Gelu")
        tiles = {}

        b1t = bpool.tile([P, S * KFS], dt.float32, tag="b1t", name="b1t")

        # --- segment 0: a merged [w1 | x-block0] head tile (ONE dma --
        # each dma instruction costs ~2us of queue latency) + one tile
        # per later x block, all on sync in consumption order; w2/b1 on
        # the parallel gpsimd queue ---
        blocks0 = blocks_by_seg[0]
        HW0 = KH * (FFS + nb0) // 2          # head0 width in bf16 cols
        head0 = bpool.tile([P, HW0], dt.bfloat16, tag="head0", name="head0")

        def _nbp(nb):
            return nb + (nb & 1)             # even per-k stride for fp8

        xts = [None] + [bpool.tile([P, KH * _nbp(nb) // 2], dt.bfloat16,
                                   tag=f"xt{bi}", name=f"xt{bi}")
                        for bi, (t0, nb) in enumerate(blocks0[1:], 1)]

        def emit_seg0():
            nc.sync.dma_start(out=head0[:, :], in_=xw[:, :HW0])
            o = HW0
            emit_w2(0)
            if use_bias:
                nc.gpsimd.dma_start(out=b1t[:], in_=b1p[:, :])
            for bi in range(1, len(blocks0)):
                w = KH * _nbp(blocks0[bi][1]) // 2
                nc.sync.dma_start(out=xts[bi][:, :], in_=xw[:, o:o + w])
                o += w

        def emit_w2(si):
            w2t = w2pool.tile([P, KFS * H], dt.bfloat16, tag="w2t",
                              name=f"w2t{si}")
            tiles[("w2", si)] = w2t
            nc.gpsimd.dma_start(
                out=w2t[:, :], in_=w2[:, si * KFS * H:(si + 1) * KFS * H])

        def lhsT2(si, w2t, k, ht):
            """GEMM2 stationary operand: w2 ht-chunk (128 cols)."""
            a = k * H + ht * P
            return w2t[:, a:a + P]

        def emit_seg(si):
            A = segs[si]
            ct = xwpool.tile([P, KH * (FFS + A)], dt.bfloat16, tag="ct",
                             name=f"ct{si}")
            tiles[("ct", si)] = ct
            nc.sync.dma_start(out=ct[:, :], in_=xw_src(seg_off[si], FFS + A))
            emit_w2(si)

        def lhsT1(si, k, ff):
            """GEMM1 stationary operand: w1 ff-chunk (128 cols)."""
            if si == 0:
                a = (k * (FFS + nb0) + ff * P) // 2
                return head0[:, a:a + P // 2].bitcast(dt.float8e3)
            ct = tiles[("ct", si)]
            A = segs[si]
            a = k * (FFS + A) + ff * P
            return ct[:, a:a + P]

        def rhs1(si, k, bi, t0, nb):
            """GEMM1 moving operand: x token block (seg0: fp8, width
            padded to even -- the extra zero column lands in PSUM and is
            ignored by the eviction)."""
            if si == 0:
                if bi == 0:
                    a = (k * (FFS + nb0) + FFS) // 2
                    return head0[:, a:a + nb // 2].bitcast(dt.float8e3)
                nbp = _nbp(blocks_by_seg[0][bi][1])
                a = k * nbp // 2
                return xts[bi][:, a:a + nbp // 2].bitcast(dt.float8e3)
            ct = tiles[("ct", si)]
            A = segs[si]
            a = k * (FFS + A) + FFS + t0
            return ct[:, a:a + nb]

        # warm-up zero tile first in the vector queue (no input deps) so
        # the PE can start ramping before any DMA lands
        warm = bpool.tile([P, NBMAX], dt.bfloat16, tag="warm", name="warm")
        nc.vector.memset(warm[:, :], 0.0)

        emit_seg0()
        if S > 1:
            emit_seg(1)

        def zero_pad(ps, nb, n):
            """n zero matmuls accumulated into a PSUM group (busy filler).

            Full NBMAX width regardless of nb: wider zeros cover more
            wall-clock per instruction; the real accumulation only reads
            ps[:, :nb].
            """
            for i in range(n):
                nc.tensor.matmul(
                    ps[:, :NBMAX],
                    lhsT=warm[:, :P],
                    rhs=warm[:, :NBMAX],
                    start=(i == 0),
                    stop=False,
                )

        def g1_block(si, hblk, bi, t0, nb):
            """GEMM1 for one token block -> hblk[:, :, t0:t0+nb]."""
            for ff in range(KFS):
                pa = psA.tile([P, NBMAX], dt.float32, tag="pa",
                              name=f"pa{si}_{bi}_{ff}")
                warm_n = 0
                if si == 0 and bi == 0 and ff == 0:
                    # Pre-warm: accumulate zero-matmuls into the first
                    # PSUM group while the first input DMAs land; also
                    # ramps the PE clock out of its cold p-state.
                    warm_n = WARM_N
                elif si == 0:
                    warm_n = BRIDGE.get(("g1", bi, ff), 0)
                zero_pad(pa, nb, warm_n)
                nbw = _nbp(nb) if si == 0 else nb
                for k in range(KH):
                    nc.tensor.matmul(
                        pa[:, :nbw],
                        lhsT=lhsT1(si, k, ff),
                        rhs=rhs1(si, k, bi, t0, nb),
                        start=(k == 0 and warm_n == 0),
                        stop=(k == KH - 1),
                    )
                dscale = 1.0 / (SW0 * SX0) if si == 0 else 1.0
                if use_bias:
                    nc.scalar.activation(
                        hblk[:, ff, t0:t0 + nb],
                        pa[:, :nb],
                        act,
                        bias=b1t[:, si * KFS + ff:si * KFS + ff + 1],
                        scale=dscale,
                    )
                else:
                    nc.scalar.activation(
                        hblk[:, ff, t0:t0 + nb],
                        pa[:, :nb],
                        act,
                        scale=dscale,
                    )

        def g2_block(si, w2t, hblk, bi, t0, nb, last_seg_blocks):
            """GEMM2 for one token block -> packed y DMA."""
            boff = boffs[(si, bi)]
            # ot is PACKED at stride nb so the output DMA is one
            # contiguous [ht][t] run per partition (large descriptors)
            ot = opool.tile([P, KH * NBMAX], dt.bfloat16, tag="ot",
                            name=f"ot{si}_{bi}")
            # tail: the last two blocks drain in staggered row-chunks
            # so only a small transfer trails the final matmul
            nblk_left = last_seg_blocks - bi if si == S - 1 else 99
            if nblk_left == 1:       # final block: 2-ht then 1-ht chunks
                stagger = {1: 0, 3: 2, 5: 4, 6: 6, 7: 7}
            elif nblk_left == 2:     # second-to-last: 4-ht chunks
                stagger = {3: 0, 7: 4}
            else:
                stagger = None
            for ht in range(KH):
                pb = psB.tile([P, NBMAX], dt.float32, tag="pb",
                              name=f"pb{si}_{bi}_{ht}")
                warm_n = BRIDGE.get(("g2", bi, ht), 0) if si == 0 else 0
                zero_pad(pb, nb, warm_n)
                for k in range(KFS):
                    nc.tensor.matmul(
                        pb[:, :nb],
                        lhsT=lhsT2(si, w2t, k, ht),
                        rhs=hblk[:, k, t0:t0 + nb],
                        start=(k == 0 and warm_n == 0),
                        stop=(k == KFS - 1),
                    )
                nc.vector.tensor_copy(ot[:, ht * nb:(ht + 1) * nb],
                                      pb[:, :nb])
                if stagger is not None and ht in stagger:
                    lo = stagger[ht]
                    nc.sync.dma_start(
                        out=y[:, boff + lo * nb:boff + (ht + 1) * nb],
                        in_=ot[:, lo * nb:(ht + 1) * nb],
                    )
            if stagger is None:
                nc.sync.dma_start(
                    out=y[:, boff:boff + KH * nb],
                    in_=ot[:, :KH * nb],
                )

        for si, A in enumerate(segs):
            blocks = blocks_by_seg[si]
            nblk = len(blocks)
            hblk = hpool.tile([P, KFS, A], dt.bfloat16, tag="hblk",
                              name=f"hblk{si}")
            if si == 0:
                # Segment 0 is DMA-arrival-paced: interleave GEMM2 blocks
                # one behind GEMM1 (lag-1) so the PE has ~2x compute per
                # input byte while the head transfers land.
                w2t = tiles.pop(("w2", 0))
                for i in range(nblk + 2):
                    if i >= 2:
                        t0, nb = blocks[i - 2]
                        g2_block(0, w2t, hblk, i - 2, t0, nb, nblk)
                    if i < nblk:
                        t0, nb = blocks[i]
                        g1_block(0, hblk, i, t0, nb)
            else:
                for bi, (t0, nb) in enumerate(blocks):
                    g1_block(si, hblk, bi, t0, nb)
                    if bi == 0 and si + 1 < S:
                        # Prefetch segment si+1 while the rest of this
                        # segment computes (~28us of cover for ~4MB).
                        emit_seg(si + 1)
                w2t = tiles.pop(("w2", si))
                for bi, (t0, nb) in enumerate(blocks):
                    g2_block(si, w2t, hblk, bi, t0, nb, nblk)

    nc.compile()
    return nc


def _get_program(segs: tuple[int, ...], use_bias: bool = True):
    key = (segs, use_bias)
    if key not in _prog_cache:
        _prog_cache[key] = _build_program(segs, use_bias)
    return _prog_cache[key]


def _route(xf: np.ndarray, Wg: np.ndarray, bg: np.ndarray):
    """fp32 gate: softmax -> top-2 (stable order, matches jax top_k) -> renorm."""
    logits = xf @ np.asarray(Wg, np.float32) + np.asarray(bg, np.float32)
    m = logits.max(axis=1, keepdims=True)
    p = np.exp(logits - m, dtype=np.float32)
    p /= p.sum(axis=1, keepdims=True)
    order = np.argsort(-p, axis=1, kind="stable")
    idx = order[:, :TOPK]
    pv = np.take_along_axis(p, idx, axis=1)
    vals = (pv / pv.sum(axis=1, keepdims=True)).astype(np.float32)
    return idx, vals


def _pack_pm(arr_hc: np.ndarray) -> np.ndarray:
    """[H, C] -> partition-major [P, KH*C] (per partition: [k][c])."""
    h, c = arr_hc.shape
    return np.ascontiguousarray(
        arr_hc.reshape(h // P, P, c).transpose(1, 0, 2).reshape(P, -1)
    )


def kernel(x, Wg, bg, W1, b1, W2, b2):
    global LAST_RESULTS, LAST_CALL
    from concourse.bass_utils import run_bass_kernel_spmd

    bf16 = ml_dtypes.bfloat16
    x = np.asarray(x, np.float32)
    xf = x.reshape(-1, H)
    T = xf.shape[0]

    idx, vals = _route(xf, Wg, bg)
    counts = np.bincount(idx.ravel(), minlength=E)

    # Segments: experts by token count (desc), zero-count experts skipped.
    order = [int(e) for e in np.argsort(-counts, kind="stable") if counts[e] > 0]
    segs = tuple(int(counts[e]) for e in order)
    S = len(segs)
    Ctot = sum(segs)
    A0 = segs[0]
    nb0 = min(NB0, A0)
    blocks_by_seg, boffs = _plan(segs)

    use_bias = bool(np.any(np.asarray(b1, np.float32)))
    nc = _get_program(segs, use_bias)

    W1 = np.asarray(W1, np.float32)
    W2 = np.asarray(W2, np.float32)
    b1 = np.asarray(b1, np.float32)

    # Token ids / combine scales / packed x^T per segment (shared by cores).
    shards = []
    e3 = ml_dtypes.float8_e3m4
    xparts = []   # per segment: [P, KH*A] partition-major (seg0: fp8)
    for si in range(S):
        e = order[si]
        sel = idx == e                  # [T, 2]; at most one True per row
        ids = np.nonzero(sel.any(axis=1))[0]
        sc = vals[sel]                  # row-major => aligned with ids
        shards.append((ids, sc))
        if si == 0:
            xq = np.clip(xf[ids].T * SX0, -15.5, 15.5).astype(e3)
            xparts.append(_pack_pm(xq))
        else:
            xparts.append(_pack_pm(xf[ids].T.astype(bf16)))

    in_maps = []
    for c in range(NC):
        pieces = []
        for si in range(S):
            e = order[si]
            if si == 0:
                # fp8 pieces, byte-packed into the bf16 tensor: merged
                # [k][(w1*SW0 | xb0*SX0)] head, then per-block x pieces
                # with per-k stride padded to even
                w1q = np.clip(
                    W1[e][:, c * FFS:(c + 1) * FFS] * SW0,
                    -15.5, 15.5).astype(e3)
                x3 = xparts[0].reshape(P, KH, A0)
                w13 = _pack_pm(w1q).reshape(P, KH, FFS)
                hp = np.concatenate([w13, x3[:, :, :nb0]], axis=2)
                pieces.append(np.ascontiguousarray(hp).reshape(P, -1)
                              .view(bf16))
                for t0, nb in blocks_by_seg[0][1:]:
                    xp = x3[:, :, t0:t0 + nb]
                    if nb & 1:
                        pad = np.zeros((P, KH, 1), e3)
                        xp = np.concatenate([xp, pad], axis=2)
                    pieces.append(np.ascontiguousarray(xp).reshape(P, -1)
                                  .view(bf16))
            else:
                # per partition: [k][w1 cols | x cols] contiguous
                w1s = W1[e][:, c * FFS:(c + 1) * FFS].astype(bf16)
                w13 = _pack_pm(w1s).reshape(P, KH, FFS)
                x3 = xparts[si].reshape(P, KH, segs[si])
                pieces.append(np.concatenate([w13, x3], axis=2)
                              .reshape(P, -1))
        xwc = np.ascontiguousarray(np.concatenate(pieces, axis=1))
        w2c = np.concatenate(
            [_pack_pm(W2[order[si]][c * FFS:(c + 1) * FFS, :].astype(bf16))
             for si in range(S)],
            axis=1,
        )
        b1c = np.ascontiguousarray(np.stack(
            [b1[order[si]][c * FFS + f * P:c * FFS + (f + 1) * P]
             for si in range(S) for f in range(KFS)],
            axis=1,
        ))
        in_maps.append({"xw": xwc, "w2": np.ascontiguousarray(w2c), "b1p": b1c})

    LAST_CALL = (nc, in_maps)
    LAST_RESULTS = run_bass_kernel_spmd(nc, in_maps, list(range(NC)),
                                        trace=TRACE)

    # Sum partials across cores in the packed layout, then unpack.
    ysum2 = np.zeros((P, KH * Ctot), np.float32)
    for c in range(NC):
        ysum2 += LAST_RESULTS.results[c]["y"].astype(np.float32)
    ysum = np.empty((H, Ctot), np.float32)
    soff = 0
    for si in range(S):
        for bi, (t0, nb) in enumerate(blocks_by_seg[si]):
            boff = boffs[(si, bi)]
            blk = ysum2[:, boff:boff + KH * nb].reshape(P, KH, nb)
            ysum[:, soff + t0:soff + t0 + nb] = (
                blk.transpose(1, 0, 2).reshape(H, nb))
        soff += segs[si]

    out = np.zeros((T, H), np.float32)
    c0 = 0
    for si in range(S):
        ids, sc = shards[si]
        out[ids] += ysum[:, c0:c0 + ids.size].T * sc[:, None]
        c0 += segs[si]

    b2 = np.asarray(b2, np.float32)
    out += vals[:, 0:1] * b2[idx[:, 0]] + vals[:, 1:2] * b2[idx[:, 1]]
    return out.reshape(x.shape)


# revision 34
# speedup vs baseline: 1.0414x; 1.0012x over previous
"""FF-sharded MoE FFN kernel for Trainium2 (8 NeuronCores), v2 "W8".

Strategy (pure FF-tensor-parallel, single group):
  - Host computes the gate in fp32 (softmax -> top-2 -> renormalize).
  - Every core processes ALL routed (expert, token) visits; the FFN
    hidden dim (FF=4096) is sharded 8 ways: core c holds columns
    [c*512, (c+1)*512) of every expert's W1 and the matching rows of
    W2, and computes
        Ypart = gelu(X @ W1[:, shard] + b1[shard]) @ W2[shard, :]
    for each expert segment. The host sums the 8 partials, applies the
    top-2 combine weights, and adds the b2 term.
  - Why: per-core work is exactly sum(counts)/8 * H * FFS MAC columns
    for ANY routing - zero load imbalance and zero slot padding (the
    previous expert-pairing scheme padded ~1%). HBM traffic is
    ~50MB/core (16 W + 17 x + 17 y), hidden under ~265us of matmul.

Per-core schedule (8 segments = experts, descending token count):
  Inputs are packed PARTITION-MAJOR: per SBUF partition, each DMA'd
  piece is one contiguous [k][col] run, so every transfer is 128 large
  descriptors (small strided descriptors measured as low as 37GB/s;
  large ones ~245GB/s). Queue assignment is driven by measured queue
  rates: the sync queue is the fast one, so ALL latency-critical
  input (w1|x) and output (y) traffic goes to sync in exact
  consumption order; w2 and b1 (needed one GEMM-phase later) ride the
  parallel gpsimd SW-DGE queue. Every dma_start costs ~2-6us of
  queue-side latency before data flows, so segment 0 uses ONE merged
  [w1 | x-block0] head DMA (~1.5MB, first data ~13us) plus one DMA per
  later x block; GEMM2 blocks interleave one-behind GEMM1 (lag-1) so
  the PE has ~2x compute per input byte while transfers land.
  Zero-matmul warm-up (NBMAX-wide, accumulated ahead of the first real
  PSUM group) covers the initial DMA wait; it must keep the PE busy
  CONTIGUOUSLY ~2 aligned 3.4us HAM windows or the PE stays at half
  clock (any idle gap resets the window).
  All GEMMs bf16 on the PE with fp32 PSUM accumulation; exact gelu is
  fused into the GEMM1 PSUM eviction (ScalarE) with the b1 bias; GEMM2
  evictions (VectorE) write bf16 into a per-block PACKED staging tile
  so the y output DMA is one contiguous run per partition. The last
  two blocks drain in staggered 2/4-row-chunk DMAs so only ~0.25MB of
  transfer trails the final matmul.
"""

import sys

if "/opt/trn_rl_repo" not in sys.path:
    sys.path.insert(0, "/opt/trn_rl_repo")

import numpy as np
import ml_dtypes

H = 1024          # hidden size
E = 8             # experts
TOPK = 2
FF = 4 * H        # expert hidden dim
P = 128           # SBUF partitions
NC = 8            # cores == FF shards
FFS = FF // NC    # per-core FF shard (512)
KH = H // P       # 8  contraction chunks for GEMM1
KFS = FFS // P    # 4  contraction chunks for GEMM2 (shard)
NB0 = 320         # segment-0 head block width

_prog_cache: dict[tuple, object] = {}
LAST_RESULTS = None  # BassKernelResults of the most recent run (for test harness)
TRACE = False        # test harness can set kernel.TRACE = True for profiling
ACT_OVERRIDE = None  # sim-only: CoreSim lacks Gelu; tests may set e.g. "Relu"
LAST_CALL = None     # (nc, in_maps) of the most recent run, for re-runs
WARM_N = 17          # HAM/pstate pre-warm zero-matmuls at kernel start.
# Zero matmuls are NBMAX wide; they accumulate into the first PSUM group
# ahead of the real contraction. They must keep the PE busy CONTIGUOUSLY
# until the merged head DMA lands (~16us): HAM only flips to full clock
# after ~2 aligned 3.4us windows of uninterrupted busy.
# (phase, bi, group) -> zero MMs prepended to that group (stall bridging).
BRIDGE = {}
# Segment 0's GEMM1 inputs (w1 + x) ship as fp8 e3m4 packed BYTE-WISE
# inside the bf16 xw tensor (marshals byte-exact); SBUF APs bitcast to
# fp8 at the matmuls. Halves the critical head DMA. Values pre-scaled
# into e3m4 range; the product scale is divided out by the gelu
# eviction's input scale. Odd block widths pad per-k stride to even
# (the matmul computes one ignored zero column).
SW0 = 16.0           # W1 scale (std 0.02 -> 0.32)
SX0 = 2.0            # x scale (std 1 -> 2)


def _seg_blocks(A: int, first: int | None = None):
    """Split A token columns into near-equal blocks <= 512.

    first: width of the first TWO blocks (segment 0 only): small head
    blocks let the first GEMMs start as soon as ~0.8MB of input has
    landed. Avoid blocks < ~230: below that LDWEIGHTS (~114ns) stops
    hiding behind the matmul stream.
    """
    blocks = []
    t = 0
    if first is not None:
        nb = min(first, A)
        blocks.append((t, nb))
        t = nb
        A -= nb
    if A > 0:
        nblk = -(-A // 512)
        base = A // nblk
        rem = A % nblk
        for i in range(nblk):
            nb = base + (1 if i < rem else 0)
            blocks.append((t, nb))
            t += nb
    return blocks


def _plan(segs: tuple[int, ...]):
    """Shared host/kernel plan: per-segment blocks + packed-y offsets.

    Returns (blocks_by_seg, boffs) where boffs[(si, bi)] is the element
    offset (per partition) of that block's [ht][t]-contiguous span in
    the packed y output.
    """
    nb0 = min(NB0, segs[0])
    blocks_by_seg = [
        _seg_blocks(A, first=nb0 if si == 0 else None)
        for si, A in enumerate(segs)
    ]
    boffs = {}
    off = 0
    for si, blocks in enumerate(blocks_by_seg):
        for bi, (t0, nb) in enumerate(blocks):
            boffs[(si, bi)] = off
            off += KH * nb
    return blocks_by_seg, boffs


def _build_program(segs: tuple[int, ...], use_bias: bool = True):
    """Build + compile the per-core SPMD Bass program.

    segs: token count per segment, descending (exact per-expert counts;
    identical on all cores).

    DRAM I/O (S = len(segs), Ctot = sum(segs)):
      xw  [P, 8*(S*FFS + Ctot)] bf16  partition-major packed inputs:
          per partition, per piece: [k][cols] contiguous (segment 0 is
          stored as separate pieces: w1c0 | xblk0 | w1c1 | w1c2.. |
          xblk1 | xblk2 ..)
      w2  [P, S*KFS*H] bf16  partition-major W2 shards
      b1p [P, S*KFS]  f32   b1 shard, col si*KFS+f = b1[f*128:(f+1)*128]
      y   [P, KH*Ctot] bf16 partial YT, packed per block: each block's
          span is [ht][t] contiguous per partition (host sums cores,
          then unpacks)
    """
    from contextlib import ExitStack

    from concourse import bacc
    import concourse.mybir as mybir
    import concourse.tile as tile

    dt = mybir.dt
    S = len(segs)
    Ctot = sum(segs)
    A0 = segs[0]
    nb0 = min(NB0, A0)
    blocks_by_seg, boffs = _plan(segs)
    NBMAX = max(nb for blocks in blocks_by_seg for _, nb in blocks)

    nc = bacc.Bacc(None, target_bir_lowering=False, debug=False)

    _b0 = blocks_by_seg[0]
    _seg0w = KH * (FFS + _b0[0][1]) // 2 + sum(
        KH * (nb + (nb & 1)) // 2 for _, nb in _b0[1:])
    xw = nc.dram_tensor(
        "xw", [P, _seg0w + KH * ((S - 1) * FFS + Ctot - A0)],
        dt.bfloat16, kind="ExternalInput")
    w2 = nc.dram_tensor("w2", [P, S * KFS * H], dt.bfloat16,
                        kind="ExternalInput")
    b1p = nc.dram_tensor("b1p", [P, S * KFS], dt.float32, kind="ExternalInput")
    y = nc.dram_tensor("y", [P, KH * Ctot], dt.bfloat16, kind="ExternalOutput")

    # xw element offset (per partition) of each segment's packed block;
    # segment 0 occupies [0, KH*(FFS+A0)) split into its pieces.
    blocks0_ = blocks_by_seg[0]
    seg0_cols = KH * (FFS + blocks0_[0][1]) // 2 + sum(
        KH * (nb + (nb & 1)) // 2 for _, nb in blocks0_[1:])
    seg_off = [None, seg0_cols]
    for A in segs[1:]:
        seg_off.append(seg_off[-1] + KH * (FFS + A))

    def xw_src(elem_off: int, ncols: int):
        """2D contiguous per-partition run of xw (128 big descriptors --
        3D APs here cost ~2.6us of DGE descriptor-generation per DMA)."""
        a = elem_off
        return xw[:, a:a + KH * ncols]

    with ExitStack() as ctx:
        tc = ctx.enter_context(tile.TileContext(nc))
        xwpool = ctx.enter_context(tc.tile_pool(name="xwpool", bufs=2))
        w2pool = ctx.enter_context(tc.tile_pool(name="w2pool", bufs=2))
        bpool = ctx.enter_context(tc.tile_pool(name="bpool", bufs=1))
        hpool = ctx.enter_context(tc.tile_pool(name="hpool", bufs=2))
        psA = ctx.enter_context(tc.tile_pool(name="psA", bufs=4, space="PSUM"))
        psB = ctx.enter_context(tc.tile_pool(name="psB", bufs=4, space="PSUM"))
        opool = ctx.enter_context(tc.tile_pool(name="opool", bufs=3))

        act = getattr(mybir.ActivationFunctionType, ACT_OVERRIDE or "Gelu")
        tiles = {}

        b1t = bpool.tile([P, S * KFS], dt.float32, tag="b1t", name="b1t")

        # --- segment 0: a merged [w1 | x-block0] head tile (ONE dma --
        # each dma instruction costs ~2us of queue latency) + one tile
        # per later x block, all on sync in consumption order; w2/b1 on
        # the parallel gpsimd queue ---
        blocks0 = blocks_by_seg[0]
        HW0 = KH * (FFS + nb0) // 2          # head0 width in bf16 cols
        head0 = bpool.tile([P, HW0], dt.bfloat16, tag="head0", name="head0")

        def _nbp(nb):
            return nb + (nb & 1)             # even per-k stride for fp8

        xts = [None] + [bpool.tile([P, KH * _nbp(nb) // 2], dt.bfloat16,
                                   tag=f"xt{bi}", name=f"xt{bi}")
                        for bi, (t0, nb) in enumerate(blocks0[1:], 1)]

        def emit_seg0():
            nc.sync.dma_start(out=head0[:, :], in_=xw[:, :HW0])
            o = HW0
            emit_w2(0)
            if use_bias:
                nc.gpsimd.dma_start(out=b1t[:], in_=b1p[:, :])
            for bi in range(1, len(blocks0)):
                w = KH * _nbp(blocks0[bi][1]) // 2
                nc.sync.dma_start(out=xts[bi][:, :], in_=xw[:, o:o + w])
                o += w

        def emit_w2(si):
            w2t = w2pool.tile([P, KFS * H], dt.bfloat16, tag="w2t",
                              name=f"w2t{si}")
            tiles[("w2", si)] = w2t
            nc.gpsimd.dma_start(
                out=w2t[:, :], in_=w2[:, si * KFS * H:(si + 1) * KFS * H])

        def lhsT2(si, w2t, k, ht):
            """GEMM2 stationary operand: w2 ht-chunk (128 cols)."""
            a = k * H + ht * P
            return w2t[:, a:a + P]

        def emit_seg(si):
            A = segs[si]
            ct = xwpool.tile([P, KH * (FFS + A)], dt.bfloat16, tag="ct",
                             name=f"ct{si}")
            tiles[("ct", si)] = ct
            nc.sync.dma_start(out=ct[:, :], in_=xw_src(seg_off[si], FFS + A))
            emit_w2(si)

        def lhsT1(si, k, ff):
            """GEMM1 stationary operand: w1 ff-chunk (128 cols)."""
            if si == 0:
                a = (k * (FFS + nb0) + ff * P) // 2
                return head0[:, a:a + P // 2].bitcast(dt.float8e3)
            ct = tiles[("ct", si)]
            A = segs[si]
            a = k * (FFS + A) + ff * P
            return ct[:, a:a + P]

        def rhs1(si, k, bi, t0, nb):
            """GEMM1 moving operand: x token block (seg0: fp8, width
            padded to even -- the extra zero column lands in PSUM and is
            ignored by the eviction)."""
            if si == 0:
                if bi == 0:
                    a = (k * (FFS + nb0) + FFS) // 2
                    return head0[:, a:a + nb // 2].bitcast(dt.float8e3)
                nbp = _nbp(blocks_by_seg[0][bi][1])
                a = k * nbp // 2
                return xts[bi][:, a:a + nbp // 2].bitcast(dt.float8e3)
            ct = tiles[("ct", si)]
            A = segs[si]
            a = k * (FFS + A) + FFS + t0
            return ct[:, a:a + nb]

        # warm-up zero tile first in the vector queue (no input deps) so
        # the PE can start ramping before any DMA lands
        warm = bpool.tile([P, NBMAX], dt.bfloat16, tag="warm", name="warm")
        nc.vector.memset(warm[:, :], 0.0)

        emit_seg0()
        if S > 1:
            emit_seg(1)

        def zero_pad(ps, nb, n):
            """n zero matmuls accumulated into a PSUM group (busy filler).

            Full NBMAX width regardless of nb: wider zeros cover more
            wall-clock per instruction; the real accumulation only reads
            ps[:, :nb].
            """
            for i in range(n):
                nc.tensor.matmul(
                    ps[:, :NBMAX],
                    lhsT=warm[:, :P],
                    rhs=warm[:, :NBMAX],
                    start=(i == 0),
                    stop=False,
                )

        def g1_block(si, hblk, bi, t0, nb):
            """GEMM1 for one token block -> hblk[:, :, t0:t0+nb]."""
            for ff in range(KFS):
                pa = psA.tile([P, NBMAX], dt.float32, tag="pa",
                              name=f"pa{si}_{bi}_{ff}")
                warm_n = 0
                if si == 0 and bi == 0 and ff == 0:
                    # Pre-warm: accumulate zero-matmuls into the first
                    # PSUM group while the first input DMAs land; also
                    # ramps the PE clock out of its cold p-state.
                    warm_n = WARM_N
                elif si == 0:
                    warm_n = BRIDGE.get(("g1", bi, ff), 0)
                zero_pad(pa, nb, warm_n)
                nbw = _nbp(nb) if si == 0 else nb
                for k in range(KH):
                    nc.tensor.matmul(
                        pa[:, :nbw],
                        lhsT=lhsT1(si, k, ff),
                        rhs=rhs1(si, k, bi, t0, nb),
                        start=(k == 0 and warm_n == 0),
                        stop=(k == KH - 1),
                    )
                dscale = 1.0 / (SW0 * SX0) if si == 0 else 1.0
                if use_bias:
                    nc.scalar.activation(
                        hblk[:, ff, t0:t0 + nb],
                        pa[:, :nb],
                        act,
                        bias=b1t[:, si * KFS + ff:si * KFS + ff + 1],
                        scale=dscale,
                    )
                else:
                    nc.scalar.activation(
                        hblk[:, ff, t0:t0 + nb],
                        pa[:, :nb],
                        act,
                        scale=dscale,
                    )

        def g2_block(si, w2t, hblk, bi, t0, nb, last_seg_blocks):
            """GEMM2 for one token block -> packed y DMA."""
            boff = boffs[(si, bi)]
            # ot is PACKED at stride nb so the output DMA is one
            # contiguous [ht][t] run per partition (large descriptors)
            ot = opool.tile([P, KH * NBMAX], dt.bfloat16, tag="ot",
                            name=f"ot{si}_{bi}")
            # tail: the last two blocks drain in staggered row-chunks
            # so only a small transfer trails the final matmul
            nblk_left = last_seg_blocks - bi if si == S - 1 else 99
            if nblk_left == 1:       # final block: 2-ht then 1-ht chunks
                stagger = {1: 0, 3: 2, 5: 4, 6: 6, 7: 7}
            elif nblk_left == 2:     # second-to-last: 4-ht chunks
                stagger = {3: 0, 7: 4}
            else:
                stagger = None
            for ht in range(KH):
                pb = psB.tile([P, NBMAX], dt.float32, tag="pb",
                              name=f"pb{si}_{bi}_{ht}")
                warm_n = BRIDGE.get(("g2", bi, ht), 0) if si == 0 else 0
                zero_pad(pb, nb, warm_n)
                for k in range(KFS):
                    nc.tensor.matmul(
                        pb[:, :nb],
                        lhsT=lhsT2(si, w2t, k, ht),
                        rhs=hblk[:, k, t0:t0 + nb],
                        start=(k == 0 and warm_n == 0),
                        stop=(k == KFS - 1),
                    )
                nc.vector.tensor_copy(ot[:, ht * nb:(ht + 1) * nb],
                                      pb[:, :nb])
                if stagger is not None and ht in stagger:
                    lo = stagger[ht]
                    nc.sync.dma_start(
                        out=y[:, boff + lo * nb:boff + (ht + 1) * nb],
                        in_=ot[:, lo * nb:(ht + 1) * nb],
                    )
            if stagger is None:
                nc.sync.dma_start(
                    out=y[:, boff:boff + KH * nb],
                    in_=ot[:, :KH * nb],
                )

        for si, A in enumerate(segs):
            blocks = blocks_by_seg[si]
            nblk = len(blocks)
            hblk = hpool.tile([P, KFS, A], dt.bfloat16, tag="hblk",
                              name=f"hblk{si}")
            if si == 0:
                # Segment 0 is DMA-arrival-paced: interleave GEMM2 blocks
                # one behind GEMM1 (lag-1) so the PE has ~2x compute per
                # input byte while the head transfers land.
                w2t = tiles.pop(("w2", 0))
                for i in range(nblk + 2):
                    if i >= 2:
                        t0, nb = blocks[i - 2]
                        g2_block(0, w2t, hblk, i - 2, t0, nb, nblk)
                    if i < nblk:
                        t0, nb = blocks[i]
                        g1_block(0, hblk, i, t0, nb)
            else:
                for bi, (t0, nb) in enumerate(blocks):
                    g1_block(si, hblk, bi, t0, nb)
                    if bi == 0 and si + 1 < S:
                        # Prefetch segment si+1 while the rest of this
                        # segment computes (~28us of cover for ~4MB).
                        emit_seg(si + 1)
                w2t = tiles.pop(("w2", si))
                for bi, (t0, nb) in enumerate(blocks):
                    g2_block(si, w2t, hblk, bi, t0, nb, nblk)

    nc.compile()
    return nc


def _get_program(segs: tuple[int, ...], use_bias: bool = True):
    key = (segs, use_bias)
    if key not in _prog_cache:
        _prog_cache[key] = _build_program(segs, use_bias)
    return _prog_cache[key]


def _route(xf: np.ndarray, Wg: np.ndarray, bg: np.ndarray):
    """fp32 gate: softmax -> top-2 (stable order, matches jax top_k) -> renorm."""
    logits = xf @ np.asarray(Wg, np.float32) + np.asarray(bg, np.float32)
    m = logits.max(axis=1, keepdims=True)
    p = np.exp(logits - m, dtype=np.float32)
    p /= p.sum(axis=1, keepdims=True)
    order = np.argsort(-p, axis=1, kind="stable")
    idx = order[:, :TOPK]
    pv = np.take_along_axis(p, idx, axis=1)
    vals = (pv / pv.sum(axis=1, keepdims=True)).astype(np.float32)
    return idx, vals


def _pack_pm(arr_hc: np.ndarray) -> np.ndarray:
    """[H, C] -> partition-major [P, KH*C] (per partition: [k][c])."""
    h, c = arr_hc.shape
    return np.ascontiguousarray(
        arr_hc.reshape(h // P, P, c).transpose(1, 0, 2).reshape(P, -1)
    )


def kernel(x, Wg, bg, W1, b1, W2, b2):
    global LAST_RESULTS, LAST_CALL
    from concourse.bass_utils import run_bass_kernel_spmd

    bf16 = ml_dtypes.bfloat16
    x = np.asarray(x, np.float32)
    xf = x.reshape(-1, H)
    T = xf.shape[0]

    idx, vals = _route(xf, Wg, bg)
    counts = np.bincount(idx.ravel(), minlength=E)

    # Segments: experts by token count ASCENDING (smallest first: the
    # follow-up x pieces of the head segment are smallest, and the tail
    # segment's final block is narrowest -> smaller drain trail).
    order = [int(e) for e in np.argsort(counts, kind="stable") if counts[e] > 0]
    segs = tuple(int(counts[e]) for e in order)
    S = len(segs)
    Ctot = sum(segs)
    A0 = segs[0]
    nb0 = min(NB0, A0)
    blocks_by_seg, boffs = _plan(segs)

    use_bias = bool(np.any(np.asarray(b1, np.float32)))
    nc = _get_program(segs, use_bias)

    W1 = np.asarray(W1, np.float32)
    W2 = np.asarray(W2, np.float32)
    b1 = np.asarray(b1, np.float32)

    # Token ids / combine scales / packed x^T per segment (shared by cores).
    shards = []
    e3 = ml_dtypes.float8_e3m4
    xparts = []   # per segment: [P, KH*A] partition-major (seg0: fp8)
    for si in range(S):
        e = order[si]
        sel = idx == e                  # [T, 2]; at most one True per row
        ids = np.nonzero(sel.any(axis=1))[0]
        sc = vals[sel]                  # row-major => aligned with ids
        shards.append((ids, sc))
        if si == 0:
            xq = np.clip(xf[ids].T * SX0, -15.5, 15.5).astype(e3)
            xparts.append(_pack_pm(xq))
        else:
            xparts.append(_pack_pm(xf[ids].T.astype(bf16)))

    in_maps = []
    for c in range(NC):
        pieces = []
        for si in range(S):
            e = order[si]
            if si == 0:
                # fp8 pieces, byte-packed into the bf16 tensor: merged
                # [k][(w1*SW0 | xb0*SX0)] head, then per-block x pieces
                # with per-k stride padded to even
                w1q = np.clip(
                    W1[e][:, c * FFS:(c + 1) * FFS] * SW0,
                    -15.5, 15.5).astype(e3)
                x3 = xparts[0].reshape(P, KH, A0)
                w13 = _pack_pm(w1q).reshape(P, KH, FFS)
                hp = np.concatenate([w13, x3[:, :, :nb0]], axis=2)
                pieces.append(np.ascontiguousarray(hp).reshape(P, -1)
                              .view(bf16))
                for t0, nb in blocks_by_seg[0][1:]:
                    xp = x3[:, :, t0:t0 + nb]
                    if nb & 1:
                        pad = np.zeros((P, KH, 1), e3)
                        xp = np.concatenate([xp, pad], axis=2)
                    pieces.append(np.ascontiguousarray(xp).reshape(P, -1)
                                  .view(bf16))
            else:
                # per partition: [k][w1 cols | x cols] contiguous
                w1s = W1[e][:, c * FFS:(c + 1) * FFS].astype(bf16)
                w13 = _pack_pm(w1s).reshape(P, KH, FFS)
                x3 = xparts[si].reshape(P, KH, segs[si])
                pieces.append(np.concatenate([w13, x3], axis=2)
                              .reshape(P, -1))
        xwc = np.ascontiguousarray(np.concatenate(pieces, axis=1))
        w2c = np.concatenate(
            [_pack_pm(W2[order[si]][c * FFS:(c + 1) * FFS, :].astype(bf16))
             for si in range(S)],
            axis=1,
        )
        b1c = np.ascontiguousarray(np.stack(
            [b1[order[si]][c * FFS + f * P:c * FFS + (f + 1) * P]
             for si in range(S) for f in range(KFS)],
            axis=1,
        ))
        in_maps.append({"xw": xwc, "w2": np.ascontiguousarray(w2c), "b1p": b1c})

    LAST_CALL = (nc, in_maps)
    LAST_RESULTS = run_bass_kernel_spmd(nc, in_maps, list(range(NC)),
                                        trace=TRACE)

    # Sum partials across cores in the packed layout, then unpack.
    ysum2 = np.zeros((P, KH * Ctot), np.float32)
    for c in range(NC):
        ysum2 += LAST_RESULTS.results[c]["y"].astype(np.float32)
    ysum = np.empty((H, Ctot), np.float32)
    soff = 0
    for si in range(S):
        for bi, (t0, nb) in enumerate(blocks_by_seg[si]):
            boff = boffs[(si, bi)]
            blk = ysum2[:, boff:boff + KH * nb].reshape(P, KH, nb)
            ysum[:, soff + t0:soff + t0 + nb] = (
                blk.transpose(1, 0, 2).reshape(H, nb))
        soff += segs[si]

    out = np.zeros((T, H), np.float32)
    c0 = 0
    for si in range(S):
        ids, sc = shards[si]
        out[ids] += ysum[:, c0:c0 + ids.size].T * sc[:, None]
        c0 += segs[si]

    b2 = np.asarray(b2, np.float32)
    out += vals[:, 0:1] * b2[idx[:, 0]] + vals[:, 1:2] * b2[idx[:, 1]]
    return out.reshape(x.shape)
